# revision 1
# baseline (speedup 1.0000x reference)
"""Trainium2 Bass kernel for an 8-batch single-head attention block.

Reference computation (per batch b of 8, S=2048 seq, D=A=768):
    Q = relu(X Wq + bq); K = relu(X Wk + bk); V = relu(X Wv + bv)
    P = softmax(Q K^T)          (no 1/sqrt(d) scale)
    X1 = LN(X + P V)
    X2 = LN(X1 + X1 Wd + bd)    (LN affines are identity in this problem)

Sharding: data-parallel — batch b -> NeuronCore b (8 cores, no collectives).

Per-core plan. Matmuls run in fp32r (the TF32-like PE mode, ~19-bit
mantissa rounding on inputs, fp32 accumulate); E^T is stored bf16 (softmax
weights tolerate it). Matmul inputs are rounded by the PSUM->SBUF
evacuation copies; weights and X^T are pre-rounded host-side.

  Phase B (per 512-col s-chunk): DMA host-transposed X^T, compute
    K^T[e,s] (kept in SBUF), Q^T[e,s] (spilled to DRAM scratch), and
    V_aug[s, 0:770] (kept; col 768 == 1.0 gives softmax row-sums for free,
    col 769 zero pad for fp32r's even-width rule).
  Phase C (per 512-col q-chunk), fully fused:
    S^T = K Q^T per k-block -> E^T = exp(S^T) in bf16 (no max subtraction:
    scores < ~72 so exp stays in fp32 range); attn rows accumulate in PSUM
    with the row-sum in col 768; normalize by its reciprocal + residual +
    LayerNorm -> X1 (SBUF); PE-transpose X1 -> X1^T; X1 Wd (+bd via DVE
    broadcast add) + residual + LayerNorm -> out rows.
"""

from contextlib import ExitStack

import numpy as np

import concourse.bass as bass
import concourse.mybir as mybir
import concourse.tile as tile
from concourse import bacc
from concourse.bass_utils import run_bass_kernel_spmd
from concourse.masks import make_identity

S, D = 2048, 768
N_CORES = 8
SB, DB = S // 128, D // 128  # 16 s-blocks, 6 d-blocks
SCH = 512   # phase-B s-chunk width
QCH = 512   # phase-C q-chunk width
F32 = mybir.dt.float32
F32R = mybir.dt.float32r
BF16 = mybir.dt.bfloat16
ET_DT = BF16
V_DT = BF16
AF = mybir.ActivationFunctionType
ALU = mybir.AluOpType
EPS = 1e-5


def _round_fp32r(x: np.ndarray) -> np.ndarray:
    """Round to the PE's 19-bit fp32r format (round-to-nearest)."""
    b = np.ascontiguousarray(x, dtype=np.float32).view(np.uint32).copy()
    rem = b & np.uint32(0xFFF)
    out = b & np.uint32(0xFFFFF000)
    up = (rem > 0x800) | ((rem == 0x800) & (((b >> 12) & 1) == 1))
    return (out + (up.astype(np.uint32) << 12)).view(np.float32)


def _split_matmul_waits(nc):
    """Walrus allows only one semaphore wait on self-loading (fp32/fp32r/
    transpose) Matmult instructions; move extra waits onto a preceding
    InstEventSemaphore (which may carry two waits each)."""
    for bb in nc.main_func.blocks:
        new_insts = []
        for inst in bb.instructions:
            if isinstance(inst, mybir.InstMatmult) and inst.sync_info is not None \
                    and len(inst.sync_info.on_wait) > 1:
                waits = list(inst.sync_info.on_wait)
                extra, keep = waits[:-1], waits[-1:]
                while extra:
                    chunk, extra = extra[:2], extra[2:]
                    ev = mybir.InstEventSemaphore(
                        name=nc.get_next_instruction_name(), ins=[], outs=[])
                    ev.engine = inst.engine
                    ev.sync_info = mybir.SyncInfo(on_wait=chunk, on_update=[])
                    nc.register_instruction(ev)
                    new_insts.append(ev)
                inst.sync_info = mybir.SyncInfo(
                    on_wait=keep, on_update=list(inst.sync_info.on_update))
            new_insts.append(inst)
        bb.instructions[:] = new_insts


def _build():
    nc = bacc.Bacc("TRN2", target_bir_lowering=False, debug=False,
                   enable_asserts=False, num_devices=N_CORES)

    x_d = nc.dram_tensor("x", [S, D], F32, kind="ExternalInput").ap()
    xt_d = nc.dram_tensor("xt", [DB, 128, S], F32R, kind="ExternalInput").ap()
    wq_d = nc.dram_tensor("wq", [DB, 128, D], F32R, kind="ExternalInput").ap()
    wk_d = nc.dram_tensor("wk", [DB, 128, D], F32R, kind="ExternalInput").ap()
    wv_d = nc.dram_tensor("wv", [DB, 128, D + 2], F32R, kind="ExternalInput").ap()
    wd_d = nc.dram_tensor("wd", [DB, 128, D], F32R, kind="ExternalInput").ap()
    bqk_d = nc.dram_tensor("bqk", [128, 2 * DB], F32, kind="ExternalInput").ap()
    bv_d = nc.dram_tensor("bv", [128, D + 2], F32, kind="ExternalInput").ap()
    bd_d = nc.dram_tensor("bd", [128, D], F32, kind="ExternalInput").ap()
    out_d = nc.dram_tensor("out", [S, D], F32, kind="ExternalOutput").ap()
    qt_d = nc.dram_tensor("qt_scratch", [DB, 128, S], F32R).ap()

    with tile.TileContext(nc) as tc, ExitStack() as ctx:
        consts = ctx.enter_context(tc.tile_pool(name="consts", bufs=1))
        pers = ctx.enter_context(tc.tile_pool(name="pers", bufs=1))
        wdp = ctx.enter_context(tc.tile_pool(name="wdp", bufs=1))

        ident = consts.tile([128, 128], F32, tag="ident", name="ident")
        make_identity(nc, ident[:])
        bv_sb = consts.tile([128, D + 2], F32, tag="bv", name="bv")
        nc.scalar.dma_start(bv_sb[:], bv_d[:])
        bd_sb = consts.tile([128, D], F32, tag="bd", name="bd")
        nc.scalar.dma_start(bd_sb[:], bd_d[:])
        wd_sb = []
        for d in range(DB):
            t = wdp.tile([128, D], F32R, tag=f"wd{d}", name=f"wd{d}")
            nc.scalar.dma_start(t[:], wd_d[d])
            wd_sb.append(t)
        eps_sb = consts.tile([128, 1], F32, tag="eps", name="eps")
        nc.gpsimd.memset(eps_sb[:], EPS)
        bqk_sb = consts.tile([128, 2 * DB], F32, tag="bqk", name="bqk")
        nc.sync.dma_start(bqk_sb[:], bqk_d[:])
        bq_sb = [bqk_sb[:, e:e + 1] for e in range(DB)]
        bk_sb = [bqk_sb[:, DB + e:DB + e + 1] for e in range(DB)]

        # ---------------- Phase B: K^T (resident), Q^T (-> DRAM), V (resident)
        kt = {}    # (e, chunk) -> [128, SCH] f32r tile
        v_sb = []  # k-block -> [128, 770] f32r tile
        for k in range(SB):
            v_sb.append(pers.tile([128, D + 2], V_DT, tag=f"v{k}", name=f"v{k}"))

        with tc.tile_pool(name="wqkv", bufs=1) as wpool, \
             tc.tile_pool(name="bx", bufs=2) as bx, \
             tc.tile_pool(name="bpm", bufs=3, space="PSUM") as bpm:
            wq_sb, wk_sb, wv_sb = [], [], []
            for d in range(DB):
                t = wpool.tile([128, D], F32R, tag=f"wq{d}", name=f"wq{d}")
                nc.scalar.dma_start(t[:], wq_d[d])
                wq_sb.append(t)
                t = wpool.tile([128, D], F32R, tag=f"wk{d}", name=f"wk{d}")
                nc.scalar.dma_start(t[:], wk_d[d])
                wk_sb.append(t)
                t = wpool.tile([128, D + 2], F32R, tag=f"wv{d}", name=f"wv{d}")
                nc.scalar.dma_start(t[:], wv_d[d])
                wv_sb.append(t)

            nsb = SCH // 128  # s-blocks per chunk
            for c in range(S // SCH):
                xt_c = []
                for d in range(DB):
                    t = bx.tile([128, SCH], F32R, tag=f"xt{d}", name=f"xt{d}")
                    nc.sync.dma_start(t[:], xt_d[d, :, c * SCH:(c + 1) * SCH])
                    xt_c.append(t)
                # K^T and Q^T e-blocks
                for e in range(DB):
                    pk = bpm.tile([128, SCH], F32, tag="pmm", name="pmm")
                    for d in range(DB):
                        nc.tensor.matmul(pk[:], wk_sb[d][:, e * 128:(e + 1) * 128],
                                         xt_c[d][:], start=(d == 0), stop=(d == DB - 1))
                    kt_t = pers.tile([128, SCH], F32R, tag=f"kt{e}_{c}",
                                     name=f"kt{e}_{c}")
                    nc.scalar.activation(kt_t[:], pk[:], AF.Relu, bias=bk_sb[e])
                    kt[(e, c)] = kt_t

                    pq = bpm.tile([128, SCH], F32, tag="pmm", name="pmm")
                    for d in range(DB):
                        nc.tensor.matmul(pq[:], wq_sb[d][:, e * 128:(e + 1) * 128],
                                         xt_c[d][:], start=(d == 0), stop=(d == DB - 1))
                    qt_t = bx.tile([128, SCH], F32R, tag="qt_out", name="qt_out")
                    nc.scalar.activation(qt_t[:], pq[:], AF.Relu, bias=bq_sb[e])
                    nc.sync.dma_start(qt_d[e, :, c * SCH:(c + 1) * SCH], qt_t[:])
                # V s-blocks (col 768 == 1.0 via bv_aug for softmax row-sums)
                for sb in range(nsb):
                    k_idx = c * nsb + sb
                    for n0, nw in ((0, 512), (512, D + 2 - 512)):
                        pv = bpm.tile([128, 512], F32, tag="pmm", name="pmm")
                        for d in range(DB):
                            nc.tensor.matmul(pv[:, :nw],
                                             xt_c[d][:, sb * 128:(sb + 1) * 128],
                                             wv_sb[d][:, n0:n0 + nw],
                                             start=(d == 0), stop=(d == DB - 1))
                        vb = bx.tile([128, 512], F32, tag="vb", name="vb")
                        nc.vector.tensor_add(vb[:, :nw], pv[:, :nw],
                                             bv_sb[:, n0:n0 + nw])
                        nc.scalar.activation(v_sb[k_idx][:, n0:n0 + nw],
                                             vb[:, :nw], AF.Relu)

        # ------- Phase C (fused): scores -> exp -> attn -> LN1 -> proj -> LN2
        with tc.tile_pool(name="cx", bufs=2) as cx, \
             tc.tile_pool(name="cx1", bufs=1) as cx1, \
             tc.tile_pool(name="cet", bufs=1) as cet, \
             tc.tile_pool(name="cst", bufs=2, space="PSUM") as cst, \
             tc.tile_pool(name="cpa0", bufs=2, space="PSUM") as cpa0, \
             tc.tile_pool(name="cpa1", bufs=1, space="PSUM") as cpa1, \
             tc.tile_pool(name="cpt", bufs=1, space="PSUM") as cpt, \
             tc.tile_pool(name="cpp", bufs=2, space="PSUM") as cpp:
            nqb = QCH // 128  # q-blocks per chunk
            kt_per_chunk = SCH // 128
            for c in range(S // QCH):
                qt_c = []
                for e in range(DB):
                    t = cx.tile([128, QCH], F32R, tag=f"qt{e}", name=f"qt{e}",
                                bufs=1)
                    nc.sync.dma_start(t[:], qt_d[e, :, c * QCH:(c + 1) * QCH])
                    qt_c.append(t)
                x_res = []
                for qs in range(nqb):
                    t = cx.tile([128, D], F32, tag=f"xr{qs}", name=f"xr{qs}",
                                bufs=1)
                    nc.sync.dma_start(t[:], x_d[c * QCH + qs * 128:
                                                c * QCH + (qs + 1) * 128, :])
                    x_res.append(t)
                # E^T = exp(K Q^T) per k-block, stored bf16
                et = []
                for k in range(SB):
                    pst = cst.tile([128, QCH], F32, tag="pst", name="pst")
                    for e in range(DB):
                        nc.tensor.matmul(
                            pst[:],
                            kt[(e, k // kt_per_chunk)][
                                :, (k % kt_per_chunk) * 128:
                                   (k % kt_per_chunk + 1) * 128],
                            qt_c[e][:], start=(e == 0), stop=(e == DB - 1))
                    et_t = cet.tile([128, QCH], ET_DT, tag=f"et{k}", name=f"et{k}")
                    nc.scalar.activation(et_t[:], pst[:], AF.Exp)
                    et.append(et_t)
                # attn + rowsum -> normalize + residual -> LN1 -> X1
                x1_t = []
                for qs in range(nqb):
                    pa0 = cpa0.tile([128, 512], F32, tag="pa0", name="pa0")
                    pa1 = cpa1.tile([128, D + 2 - 512], F32, tag="pa1", name="pa1")
                    for k in range(SB):
                        nc.tensor.matmul(pa0[:],
                                         et[k][:, qs * 128:(qs + 1) * 128],
                                         v_sb[k][:, 0:512],
                                         start=(k == 0), stop=(k == SB - 1))
                    for k in range(SB):
                        nc.tensor.matmul(pa1[:],
                                         et[k][:, qs * 128:(qs + 1) * 128],
                                         v_sb[k][:, 512:D + 2],
                                         start=(k == 0), stop=(k == SB - 1))
                    rcp = cx.tile([128, 1], F32, tag="rcp", name="rcp")
                    nc.vector.reciprocal(rcp[:], pa1[:, 256:257])
                    r_t = cx.tile([128, D], F32, tag="r_t", name="r_t")
                    s0 = cx.tile([128, 1], F32, tag="s0", name="s0")
                    s1 = cx.tile([128, 1], F32, tag="s1", name="s1")
                    nc.vector.scalar_tensor_tensor(
                        r_t[:, 0:512], pa0[:], rcp[:], x_res[qs][:, 0:512],
                        op0=ALU.mult, op1=ALU.add, accum_out=s0[:])
                    nc.vector.scalar_tensor_tensor(
                        r_t[:, 512:D], pa1[:, 0:256], rcp[:], x_res[qs][:, 512:D],
                        op0=ALU.mult, op1=ALU.add, accum_out=s1[:])
                    negmu = cx.tile([128, 1], F32, tag="negmu", name="negmu")
                    nc.vector.tensor_add(negmu[:], s0[:], s1[:])
                    nc.scalar.mul(negmu[:], negmu[:], -1.0 / D)
                    sq_t = cx.tile([128, D], F32, tag="sq_t", name="sq_t",
                                   bufs=1)
                    ssq = cx.tile([128, 1], F32, tag="ssq", name="ssq")
                    nc.vector.scalar_tensor_tensor(
                        sq_t[:], r_t[:], 0.0, r_t[:],
                        op0=ALU.add, op1=ALU.mult, accum_out=ssq[:])
                    # centered ssq = ssq_raw - D*mu^2: var = ssq/D - mu^2
                    mu2 = cx.tile([128, 1], F32, tag="mu2", name="mu2")
                    nc.vector.tensor_mul(mu2[:], negmu[:], negmu[:])
                    var = cx.tile([128, 1], F32, tag="var", name="var")
                    nc.vector.scalar_tensor_tensor(
                        var[:], ssq[:], 1.0 / D, mu2[:],
                        op0=ALU.mult, op1=ALU.subtract)
                    sd = cx.tile([128, 1], F32, tag="sd", name="sd")
                    nc.scalar.activation(sd[:], var[:], AF.Sqrt,
                                         bias=eps_sb[:], scale=1.0)
                    rstd = cx.tile([128, 1], F32, tag="rstd", name="rstd")
                    nc.vector.reciprocal(rstd[:], sd[:])
                    nmr = cx.tile([128, 1], F32, tag="nmr", name="nmr")
                    nc.vector.tensor_mul(nmr[:], negmu[:], rstd[:])
                    x1 = cx1.tile([128, D], F32, tag=f"x1_{qs}", name=f"x1_{qs}")
                    nc.scalar.activation(x1[:], r_t[:], AF.Identity,
                                         bias=nmr[:], scale=rstd[:])
                    x1_t.append(x1)
                # X1^T via PE transposes
                x1t_c = [cx.tile([128, QCH], F32R, tag=f"x1t{d}", name=f"x1t{d}",
                                 bufs=1) for d in range(DB)]
                for d in range(DB):
                    for qs in range(nqb):
                        pt = cpt.tile([128, 128], F32, tag="pt", name="pt")
                        nc.tensor.transpose(
                            pt[:], x1_t[qs][:, d * 128:(d + 1) * 128], ident[:])
                        nc.vector.tensor_copy(
                            x1t_c[d][:, qs * 128:(qs + 1) * 128], pt[:])
                # proj + residual(+bd) -> LN2 -> out
                for qs in range(nqb):
                    x1bd = cx.tile([128, D], F32, tag="x1bd", name="x1bd")
                    nc.vector.tensor_add(x1bd[:], x1_t[qs][:], bd_sb[:])
                    pp = []
                    for n0, nw in ((0, 384), (384, 384)):
                        p = cpp.tile([128, 384], F32, tag="pp", name="pp")
                        for d in range(DB):
                            nc.tensor.matmul(p[:],
                                             x1t_c[d][:, qs * 128:(qs + 1) * 128],
                                             wd_sb[d][:, n0:n0 + nw],
                                             start=(d == 0), stop=(d == DB - 1))
                        pp.append(p)
                    y_t = cx.tile([128, D], F32, tag="y_t", name="y_t")
                    t0 = cx.tile([128, 1], F32, tag="t0", name="t0")
                    t1 = cx.tile([128, 1], F32, tag="t1", name="t1")
                    nc.vector.scalar_tensor_tensor(
                        y_t[:, 0:384], pp[0][:], 0.0, x1bd[:, 0:384],
                        op0=ALU.add, op1=ALU.add, accum_out=t0[:])
                    nc.vector.scalar_tensor_tensor(
                        y_t[:, 384:D], pp[1][:], 0.0, x1bd[:, 384:D],
                        op0=ALU.add, op1=ALU.add, accum_out=t1[:])
                    negmu2 = cx.tile([128, 1], F32, tag="negmu2", name="negmu2")
                    nc.vector.tensor_add(negmu2[:], t0[:], t1[:])
                    nc.scalar.mul(negmu2[:], negmu2[:], -1.0 / D)
                    sq2 = cx.tile([128, D], F32, tag="sq2", name="sq2", bufs=1)
                    ssq2 = cx.tile([128, 1], F32, tag="ssq2", name="ssq2")
                    nc.vector.scalar_tensor_tensor(
                        sq2[:], y_t[:], 0.0, y_t[:],
                        op0=ALU.add, op1=ALU.mult, accum_out=ssq2[:])
                    mu22 = cx.tile([128, 1], F32, tag="mu22", name="mu22")
                    nc.vector.tensor_mul(mu22[:], negmu2[:], negmu2[:])
                    var2 = cx.tile([128, 1], F32, tag="var2", name="var2")
                    nc.vector.scalar_tensor_tensor(
                        var2[:], ssq2[:], 1.0 / D, mu22[:],
                        op0=ALU.mult, op1=ALU.subtract)
                    sd2 = cx.tile([128, 1], F32, tag="sd2", name="sd2")
                    nc.scalar.activation(sd2[:], var2[:], AF.Sqrt,
                                         bias=eps_sb[:], scale=1.0)
                    rstd2 = cx.tile([128, 1], F32, tag="rstd2", name="rstd2")
                    nc.vector.reciprocal(rstd2[:], sd2[:])
                    nmr2 = cx.tile([128, 1], F32, tag="nmr2", name="nmr2")
                    nc.vector.tensor_mul(nmr2[:], negmu2[:], rstd2[:])
                    out_t = cx.tile([128, D], F32, tag="out_t", name="out_t")
                    nc.scalar.activation(out_t[:], y_t[:], AF.Identity,
                                         bias=nmr2[:], scale=rstd2[:])
                    r0 = c * QCH + qs * 128
                    nc.sync.dma_start(out_d[r0:r0 + 128, :], out_t[:])

    _split_matmul_waits(nc)
    nc.compile()
    return nc


_NC_CACHE = None


def _get_nc():
    global _NC_CACHE
    if _NC_CACHE is None:
        _NC_CACHE = _build()
    return _NC_CACHE


def _prep_in_maps(X, Wq, bq, Wk, bk, Wv, bv, Wd, bd):
    X = np.ascontiguousarray(X, np.float32)
    wq = _round_fp32r(Wq).reshape(DB, 128, D)
    wk = _round_fp32r(Wk).reshape(DB, 128, D)
    wv_aug = np.zeros((D, D + 2), np.float32)
    wv_aug[:, :D] = Wv
    wv_aug = _round_fp32r(wv_aug).reshape(DB, 128, D + 2)
    wd = _round_fp32r(Wd).reshape(DB, 128, D)
    bv_aug = np.zeros((1, D + 2), np.float32)
    bv_aug[0, :D] = bv
    bv_aug[0, D] = 1.0
    bv_aug = np.ascontiguousarray(np.broadcast_to(bv_aug, (128, D + 2)))
    bd_b = np.ascontiguousarray(
        np.broadcast_to(np.asarray(bd, np.float32).reshape(1, D), (128, D)))
    shared = {
        "wq": wq, "wk": wk, "wv": wv_aug, "wd": wd,
        "bqk": np.ascontiguousarray(np.concatenate(
            [np.asarray(bq, np.float32).reshape(DB, 128, 1),
             np.asarray(bk, np.float32).reshape(DB, 128, 1)], axis=0)
            .transpose(1, 0, 2).reshape(128, 2 * DB)),
        "bv": bv_aug, "bd": bd_b,
    }
    return [dict(shared, x=X[c],
                 xt=_round_fp32r(X[c].T).reshape(DB, 128, S))
            for c in range(N_CORES)]


def _run(inputs, trace=False, trace_kwargs=None):
    in_maps = _prep_in_maps(
        inputs["X"], inputs["Wq"], inputs["bq"], inputs["Wk"], inputs["bk"],
        inputs["Wv"], inputs["bv"], inputs["Wd"], inputs["bd"])
    nc = _get_nc()
    res = run_bass_kernel_spmd(nc, in_maps, list(range(N_CORES)),
                               trace=trace, **(trace_kwargs or {}))
    out = np.stack([res.results[c]["out"] for c in range(N_CORES)])
    return out, res


def kernel(X, Wq, bq, Wk, bk, Wv, bv, Wd, bd, g1, b1, g2, b2):
    out, _ = _run(dict(X=X, Wq=Wq, bq=bq, Wk=Wk, bk=bk, Wv=Wv, bv=bv,
                       Wd=Wd, bd=bd))
    g1 = np.asarray(g1); b1 = np.asarray(b1)
    g2 = np.asarray(g2); b2 = np.asarray(b2)
    # The kernel folds the (identity) LN affines away; handle the general
    # case anyway. A non-identity g1/b1 feeds the dense layer and cannot be
    # patched after the fact -> recompute on host (never hit for this
    # problem's deterministic inputs: g=1, b=0).
    if not (np.allclose(g1, 1.0) and np.allclose(b1, 0.0)):
        return _host_reference(X, Wq, bq, Wk, bk, Wv, bv, Wd, bd, g1, b1, g2, b2)
    if not (np.allclose(g2, 1.0) and np.allclose(b2, 0.0)):
        out = out * np.asarray(g2) + np.asarray(b2)
    return out.astype(np.float32)


def _host_reference(X, Wq, bq, Wk, bk, Wv, bv, Wd, bd, g1, b1, g2, b2):
    X = np.asarray(X, np.float64)
    out = np.empty_like(X)
    for c in range(X.shape[0]):
        x = X[c]
        Q = np.maximum(x @ Wq + bq, 0)
        K = np.maximum(x @ Wk + bk, 0)
        V = np.maximum(x @ Wv + bv, 0)
        Sc = Q @ K.T
        Sc -= Sc.max(-1, keepdims=True)
        E = np.exp(Sc)
        A = (E @ V) / E.sum(-1, keepdims=True)
        X1 = x + A
        X1 = (X1 - X1.mean(-1, keepdims=True)) / np.sqrt(
            X1.var(-1, keepdims=True) + EPS) * g1 + b1
        X2 = X1 + X1 @ Wd + bd
        X2 = (X2 - X2.mean(-1, keepdims=True)) / np.sqrt(
            X2.var(-1, keepdims=True) + EPS) * g2 + b2
        out[c] = X2
    return out.astype(np.float32)



# revision 2
# speedup vs baseline: 1.2953x; 1.2953x over previous
"""Trainium2 Bass kernel for an 8-batch single-head attention block.

Reference computation (per batch b of 8, S=2048 seq, D=A=768):
    Q = relu(X Wq + bq); K = relu(X Wk + bk); V = relu(X Wv + bv)
    P = softmax(Q K^T)          (no 1/sqrt(d) scale)
    X1 = LN(X + P V)
    X2 = LN(X1 + X1 Wd + bd)    (LN affines are identity in this problem)

Sharding: data-parallel — batch b -> NeuronCore b (8 cores, no collectives).

v2 design notes (from the v1 trace):
  * v1 ran all matmuls in fp32r. The PE multiplies bf16/fp16/fp32r all at
    the same FP22 internal precision and 1 cycle/row, but fp32r weight
    loads (4B) take ~213ns vs ~107ns for 16-bit — with an LDWEIGHTS per
    matmul the PE was LDW-throughput-bound (~265ns issue period vs the
    213ns stream floor). fp16 keeps fp32r-level precision (11-bit
    mantissa -> FP22 exactly) at half the LDW/DMA cost, so v2 stores every
    matmul operand in fp16 (E^T stays bf16 for exp range; V bf16 to match).
  * Q^T is kept resident in SBUF (no DRAM spill round-trip) -> kills the
    ~6us phase-boundary stall + HAM re-throttle.
  * Biases bv/bd are folded into the matmul accumulation groups as K=1
    matmuls against a ones row (removes DVE bias adds from the critical
    path); bq/bk ride the ACT relu bias port as before.
  * LayerNorm math runs on DVE via tensor_scalar (two scalar operands), a
    centered-accumulate trick gets D*var in the same pass that computes
    the residual sum; ACT only does Relu / Exp / Sqrt -> far fewer
    activation-table reloads (~1.5us each on v1).
  * Phase C is software-pipelined: per chunk, scores(c+1) is issued
    between attn(c) and transpose/dense(c) so the PE chews on scores
    while the DVE LayerNorm chain for chunk c completes.
"""

from contextlib import ExitStack

import numpy as np

import concourse.bass as bass
import concourse.mybir as mybir
import concourse.tile as tile
from concourse import bacc
from concourse.bass_utils import run_bass_kernel_spmd
from concourse.masks import make_identity

S, D = 2048, 768
N_CORES = 8
SB, DB = S // 128, D // 128  # 16 s-blocks, 6 d-blocks
SCH = 512   # phase-B s-chunk width
QCH = 512   # phase-C q-chunk width
NCH = S // SCH
F32 = mybir.dt.float32
F16 = mybir.dt.float16
BF16 = mybir.dt.bfloat16
ET_DT = BF16
V_DT = BF16
AF = mybir.ActivationFunctionType
ALU = mybir.AluOpType
EPS = 1e-5


def _split_matmul_waits(nc):
    """Walrus allows only one semaphore wait on self-loading (fp32/fp32r/
    transpose) Matmult instructions; move extra waits onto a preceding
    InstEventSemaphore (which may carry two waits each)."""
    for bb in nc.main_func.blocks:
        new_insts = []
        for inst in bb.instructions:
            if isinstance(inst, mybir.InstMatmult) and inst.sync_info is not None \
                    and len(inst.sync_info.on_wait) > 1:
                waits = list(inst.sync_info.on_wait)
                extra, keep = waits[:-1], waits[-1:]
                while extra:
                    chunk, extra = extra[:2], extra[2:]
                    ev = mybir.InstEventSemaphore(
                        name=nc.get_next_instruction_name(), ins=[], outs=[])
                    ev.engine = inst.engine
                    ev.sync_info = mybir.SyncInfo(on_wait=chunk, on_update=[])
                    nc.register_instruction(ev)
                    new_insts.append(ev)
                inst.sync_info = mybir.SyncInfo(
                    on_wait=keep, on_update=list(inst.sync_info.on_update))
            new_insts.append(inst)
        bb.instructions[:] = new_insts


def _build():
    nc = bacc.Bacc("TRN2", target_bir_lowering=False, debug=False,
                   enable_asserts=False, num_devices=N_CORES)

    xt_d = nc.dram_tensor("xt", [DB, 128, S], F16, kind="ExternalInput").ap()
    xr_d = nc.dram_tensor("xr", [S, D], F16, kind="ExternalInput").ap()
    wq_d = nc.dram_tensor("wq", [DB, 128, D], F16, kind="ExternalInput").ap()
    wk_d = nc.dram_tensor("wk", [DB, 128, D], F16, kind="ExternalInput").ap()
    wv_d = nc.dram_tensor("wv", [DB, 128, D + 2], F16, kind="ExternalInput").ap()
    wd_d = nc.dram_tensor("wd", [DB, 128, D], F16, kind="ExternalInput").ap()
    bqk_d = nc.dram_tensor("bqk", [128, 2 * DB], F32, kind="ExternalInput").ap()
    bv_d = nc.dram_tensor("bv", [1, D + 2], F16, kind="ExternalInput").ap()
    bd_d = nc.dram_tensor("bd", [1, D], F16, kind="ExternalInput").ap()
    out_d = nc.dram_tensor("out", [S, D], F32, kind="ExternalOutput").ap()

    with tile.TileContext(nc) as tc, ExitStack() as ctx:
        consts = ctx.enter_context(tc.tile_pool(name="consts", bufs=1))
        pers = ctx.enter_context(tc.tile_pool(name="pers", bufs=1))
        wdp = ctx.enter_context(tc.tile_pool(name="wdp", bufs=1))

        # critical-path DMAs first on the sync queue: bqk, wk, xt chunk 0
        bqk_sb = consts.tile([128, 2 * DB], F32, tag="bqk", name="bqk")
        nc.sync.dma_start(bqk_sb[:], bqk_d[:])
        bq_sb = [bqk_sb[:, e:e + 1] for e in range(DB)]
        bk_sb = [bqk_sb[:, DB + e:DB + e + 1] for e in range(DB)]

        ident = consts.tile([128, 128], F16, tag="ident", name="ident")
        make_identity(nc, ident[:])
        ones_sb = consts.tile([1, 128], F16, tag="ones", name="ones")
        nc.gpsimd.memset(ones_sb[:], 1.0)
        eps_sb = consts.tile([128, 1], F32, tag="eps", name="eps")
        nc.gpsimd.memset(eps_sb[:], EPS)
        bv_sb = consts.tile([1, D + 2], F16, tag="bv", name="bv")
        nc.scalar.dma_start(bv_sb[:], bv_d[:])
        bd_sb = consts.tile([1, D], F16, tag="bd", name="bd")
        nc.scalar.dma_start(bd_sb[:], bd_d[:])
        wd_sb = []
        for d in range(DB):
            t = wdp.tile([128, D], F16, tag=f"wd{d}", name=f"wd{d}")
            nc.scalar.dma_start(t[:], wd_d[d])
            wd_sb.append(t)

        # ---------------- Phase B: K^T, Q^T (both resident), V (resident)
        kt = {}    # (e, chunk) -> [128, SCH] f16 tile
        qt = {}    # (e, chunk) -> [128, SCH] f16 tile
        v_sb = []  # k-block -> [128, 770] bf16 tile
        for k in range(SB):
            v_sb.append(pers.tile([128, D + 2], V_DT, tag=f"v{k}", name=f"v{k}"))

        with tc.tile_pool(name="wqkv", bufs=1) as wpool, \
             tc.tile_pool(name="bx", bufs=2) as bx, \
             tc.tile_pool(name="bpm", bufs=3, space="PSUM") as bpm:
            wk_sb, wq_sb, wv_sb = [], [], []
            for d in range(DB):
                t = wpool.tile([128, D], F16, tag=f"wk{d}", name=f"wk{d}")
                nc.sync.dma_start(t[:], wk_d[d])
                wk_sb.append(t)
            xt_first = []
            for d in range(DB):
                t = bx.tile([128, SCH], F16, tag=f"xt{d}", name=f"xt{d}")
                nc.sync.dma_start(t[:], xt_d[d, :, 0:SCH])
                xt_first.append(t)
            for d in range(DB):
                t = wpool.tile([128, D], F16, tag=f"wq{d}", name=f"wq{d}")
                nc.sync.dma_start(t[:], wq_d[d])
                wq_sb.append(t)
            for d in range(DB):
                t = wpool.tile([128, D + 2], F16, tag=f"wv{d}", name=f"wv{d}")
                nc.sync.dma_start(t[:], wv_d[d])
                wv_sb.append(t)

            nsb = SCH // 128  # s-blocks per chunk
            for c in range(NCH):
                if c == 0:
                    xt_c = xt_first
                else:
                    xt_c = []
                    for d in range(DB):
                        t = bx.tile([128, SCH], F16, tag=f"xt{d}", name=f"xt{d}")
                        nc.sync.dma_start(t[:], xt_d[d, :, c * SCH:(c + 1) * SCH])
                        xt_c.append(t)
                # K^T and Q^T e-blocks
                for e in range(DB):
                    pk = bpm.tile([128, SCH], F32, tag="pmm", name="pmm")
                    for d in range(DB):
                        nc.tensor.matmul(pk[:], wk_sb[d][:, e * 128:(e + 1) * 128],
                                         xt_c[d][:], start=(d == 0), stop=(d == DB - 1))
                    kt_t = pers.tile([128, SCH], F16, tag=f"kt{e}_{c}",
                                     name=f"kt{e}_{c}")
                    nc.scalar.activation(kt_t[:], pk[:], AF.Relu, bias=bk_sb[e])
                    kt[(e, c)] = kt_t

                    pq = bpm.tile([128, SCH], F32, tag="pmm", name="pmm")
                    for d in range(DB):
                        nc.tensor.matmul(pq[:], wq_sb[d][:, e * 128:(e + 1) * 128],
                                         xt_c[d][:], start=(d == 0), stop=(d == DB - 1))
                    qt_t = pers.tile([128, SCH], F16, tag=f"qt{e}_{c}",
                                     name=f"qt{e}_{c}")
                    nc.scalar.activation(qt_t[:], pq[:], AF.Relu, bias=bq_sb[e])
                    qt[(e, c)] = qt_t
                # V s-blocks (col 768 == 1.0 via bv_aug for softmax row-sums;
                # bv added via a K=1 ones-row matmul into the same psum group)
                for sb in range(nsb):
                    k_idx = c * nsb + sb
                    for n0, nw in ((0, 512), (512, D + 2 - 512)):
                        pv = bpm.tile([128, 512], F32, tag="pmm", name="pmm")
                        for d in range(DB):
                            nc.tensor.matmul(pv[:, :nw],
                                             xt_c[d][:, sb * 128:(sb + 1) * 128],
                                             wv_sb[d][:, n0:n0 + nw],
                                             start=(d == 0), stop=False)
                        nc.tensor.matmul(pv[:, :nw], ones_sb[:],
                                         bv_sb[:, n0:n0 + nw],
                                         start=False, stop=True)
                        nc.scalar.activation(v_sb[k_idx][:, n0:n0 + nw],
                                             pv[:, :nw], AF.Relu)

        # ------- Phase C (pipelined): scores -> exp -> attn -> LN1 -> proj -> LN2
        nqb = QCH // 128   # q-blocks per chunk
        kt_per_chunk = SCH // 128
        with tc.tile_pool(name="cx", bufs=2) as cx, \
             tc.tile_pool(name="cxr", bufs=2) as cxr, \
             tc.tile_pool(name="cx1", bufs=2) as cx1, \
             tc.tile_pool(name="cx1t", bufs=2) as cx1t, \
             tc.tile_pool(name="cet", bufs=2) as cet, \
             tc.tile_pool(name="cst", bufs=2, space="PSUM") as cst, \
             tc.tile_pool(name="cpa0", bufs=2, space="PSUM") as cpa0, \
             tc.tile_pool(name="cpa1", bufs=1, space="PSUM") as cpa1, \
             tc.tile_pool(name="cpt", bufs=1, space="PSUM") as cpt, \
             tc.tile_pool(name="cpp", bufs=2, space="PSUM") as cpp:

            def load_xres(c):
                x_res = []
                for qs in range(nqb):
                    t = cxr.tile([128, D], F16, tag=f"xr{qs}", name=f"xr{qs}")
                    nc.scalar.dma_start(t[:], xr_d[c * QCH + qs * 128:
                                                   c * QCH + (qs + 1) * 128, :])
                    x_res.append(t)
                return x_res

            def stage_scores(c):
                """S^T = K Q^T per k-block -> E^T = exp(S^T) bf16 (no max
                subtraction: scores < ~72 so exp stays in fp32 range)."""
                et = []
                for k in range(SB):
                    pst = cst.tile([128, QCH], F32, tag="pst", name="pst")
                    for e in range(DB):
                        nc.tensor.matmul(
                            pst[:],
                            kt[(e, k // kt_per_chunk)][
                                :, (k % kt_per_chunk) * 128:
                                   (k % kt_per_chunk + 1) * 128],
                            qt[(e, c)][:], start=(e == 0), stop=(e == DB - 1))
                    et_t = cet.tile([128, QCH], ET_DT, tag=f"et{k}", name=f"et{k}")
                    nc.scalar.activation(et_t[:], pst[:], AF.Exp)
                    et.append(et_t)
                return et

            def stage_attn(c, et, x_res):
                """attn rows + row-sum (col 768) in PSUM; normalize + residual
                + LayerNorm -> X1 (fp16, SBUF)."""
                x1_t = []
                for qs in range(nqb):
                    pa0 = cpa0.tile([128, 512], F32, tag="pa0", name="pa0")
                    pa1 = cpa1.tile([128, D + 2 - 512], F32, tag="pa1", name="pa1")
                    for k in range(SB):
                        nc.tensor.matmul(pa0[:],
                                         et[k][:, qs * 128:(qs + 1) * 128],
                                         v_sb[k][:, 0:512],
                                         start=(k == 0), stop=(k == SB - 1))
                    for k in range(SB):
                        nc.tensor.matmul(pa1[:],
                                         et[k][:, qs * 128:(qs + 1) * 128],
                                         v_sb[k][:, 512:D + 2],
                                         start=(k == 0), stop=(k == SB - 1))
                    rcp = cx.tile([128, 1], F32, tag="rcp", name="rcp")
                    nc.vector.reciprocal(rcp[:], pa1[:, 256:257])
                    r_t = cx.tile([128, D], F32, tag="r_t", name="r_t")
                    s0 = cx.tile([128, 1], F32, tag="s0", name="s0")
                    s1 = cx.tile([128, 1], F32, tag="s1", name="s1")
                    nc.vector.scalar_tensor_tensor(
                        r_t[:, 0:512], pa0[:], rcp[:], x_res[qs][:, 0:512],
                        op0=ALU.mult, op1=ALU.add, accum_out=s0[:])
                    nc.vector.scalar_tensor_tensor(
                        r_t[:, 512:D], pa1[:, 0:256], rcp[:], x_res[qs][:, 512:D],
                        op0=ALU.mult, op1=ALU.add, accum_out=s1[:])
                    negmu = cx.tile([128, 1], F32, tag="negmu", name="negmu")
                    nc.vector.tensor_add(negmu[:], s0[:], s1[:])
                    nc.vector.tensor_scalar_mul(negmu[:], negmu[:], -1.0 / D)
                    # centered accumulate: sum((r-mu)*r) = D*var
                    sq_t = cx.tile([128, D], F32, tag="sq_t", name="sq_t", bufs=1)
                    dvar = cx.tile([128, 1], F32, tag="dvar", name="dvar")
                    nc.vector.scalar_tensor_tensor(
                        sq_t[:], r_t[:], negmu[:], r_t[:],
                        op0=ALU.add, op1=ALU.mult, accum_out=dvar[:])
                    sd = cx.tile([128, 1], F32, tag="sd", name="sd")
                    nc.scalar.activation(sd[:], dvar[:], AF.Sqrt,
                                         bias=eps_sb[:], scale=1.0 / D)
                    rstd = cx.tile([128, 1], F32, tag="rstd", name="rstd")
                    nc.vector.reciprocal(rstd[:], sd[:])
                    x1 = cx1.tile([128, D], F16, tag=f"x1_{qs}", name=f"x1_{qs}")
                    nc.vector.tensor_scalar(x1[:], r_t[:], negmu[:], rstd[:],
                                            op0=ALU.add, op1=ALU.mult)
                    x1_t.append(x1)
                return x1_t

            def stage_tdense(c, x1_t):
                # X1^T via PE transposes (fp16), 4 q-blocks batched per psum
                # bank -> one DVE copy per d-block
                x1t_c = []
                for d in range(DB):
                    pt = cpt.tile([128, 512], F16, tag="pt", name="pt")
                    for qs in range(nqb):
                        nc.tensor.transpose(
                            pt[:, qs * 128:(qs + 1) * 128],
                            x1_t[qs][:, d * 128:(d + 1) * 128], ident[:])
                    xt_t = cx1t.tile([128, QCH], F16, tag=f"x1t{d}",
                                     name=f"x1t{d}")
                    nc.vector.tensor_copy(xt_t[:], pt[:])
                    x1t_c.append(xt_t)
                # proj (+bd via K=1 ones-row matmul) -> LN2 -> out rows
                for qs in range(nqb):
                    pp = []
                    for n0, nw in ((0, 384), (384, 384)):
                        p = cpp.tile([128, 384], F32, tag="pp", name="pp")
                        for d in range(DB):
                            nc.tensor.matmul(p[:],
                                             x1t_c[d][:, qs * 128:(qs + 1) * 128],
                                             wd_sb[d][:, n0:n0 + nw],
                                             start=(d == 0), stop=False)
                        nc.tensor.matmul(p[:], ones_sb[:], bd_sb[:, n0:n0 + nw],
                                         start=False, stop=True)
                        pp.append(p)
                    y_t = cx.tile([128, D], F32, tag="y_t", name="y_t")
                    t0 = cx.tile([128, 1], F32, tag="t0", name="t0")
                    t1 = cx.tile([128, 1], F32, tag="t1", name="t1")
                    nc.vector.scalar_tensor_tensor(
                        y_t[:, 0:384], pp[0][:], 0.0, x1_t[qs][:, 0:384],
                        op0=ALU.add, op1=ALU.add, accum_out=t0[:])
                    nc.vector.scalar_tensor_tensor(
                        y_t[:, 384:D], pp[1][:], 0.0, x1_t[qs][:, 384:D],
                        op0=ALU.add, op1=ALU.add, accum_out=t1[:])
                    negmu2 = cx.tile([128, 1], F32, tag="negmu2", name="negmu2")
                    nc.vector.tensor_add(negmu2[:], t0[:], t1[:])
                    nc.vector.tensor_scalar_mul(negmu2[:], negmu2[:], -1.0 / D)
                    sq2 = cx.tile([128, D], F32, tag="sq2", name="sq2", bufs=1)
                    dvar2 = cx.tile([128, 1], F32, tag="dvar2", name="dvar2")
                    nc.vector.scalar_tensor_tensor(
                        sq2[:], y_t[:], negmu2[:], y_t[:],
                        op0=ALU.add, op1=ALU.mult, accum_out=dvar2[:])
                    sd2 = cx.tile([128, 1], F32, tag="sd2", name="sd2")
                    nc.scalar.activation(sd2[:], dvar2[:], AF.Sqrt,
                                         bias=eps_sb[:], scale=1.0 / D)
                    rstd2 = cx.tile([128, 1], F32, tag="rstd2", name="rstd2")
                    nc.vector.reciprocal(rstd2[:], sd2[:])
                    out_t = cx.tile([128, D], F32, tag="out_t", name="out_t")
                    nc.vector.tensor_scalar(out_t[:], y_t[:], negmu2[:],
                                            rstd2[:], op0=ALU.add, op1=ALU.mult)
                    r0 = c * QCH + qs * 128
                    nc.sync.dma_start(out_d[r0:r0 + 128, :], out_t[:])

            # software pipeline: scores(c+1) issues between attn(c) and
            # tdense(c) so the PE stays fed while chunk c's LN chain runs
            xres_cur = load_xres(0)
            et_cur = stage_scores(0)
            for c in range(NCH):
                if c + 1 < NCH:
                    xres_nxt = load_xres(c + 1)
                x1_t = stage_attn(c, et_cur, xres_cur)
                if c + 1 < NCH:
                    et_cur = stage_scores(c + 1)
                    xres_cur = xres_nxt
                stage_tdense(c, x1_t)

    _split_matmul_waits(nc)
    nc.compile()
    return nc


_NC_CACHE = None


def _get_nc():
    global _NC_CACHE
    if _NC_CACHE is None:
        _NC_CACHE = _build()
    return _NC_CACHE


def _prep_in_maps(X, Wq, bq, Wk, bk, Wv, bv, Wd, bd):
    X = np.ascontiguousarray(X, np.float32)
    f16 = np.float16
    wq = np.asarray(Wq, np.float32).astype(f16).reshape(DB, 128, D)
    wk = np.asarray(Wk, np.float32).astype(f16).reshape(DB, 128, D)
    wv_aug = np.zeros((D, D + 2), f16)
    wv_aug[:, :D] = np.asarray(Wv, np.float32).astype(f16)
    wv_aug = wv_aug.reshape(DB, 128, D + 2)
    wd = np.asarray(Wd, np.float32).astype(f16).reshape(DB, 128, D)
    bv_aug = np.zeros((1, D + 2), f16)
    bv_aug[0, :D] = np.asarray(bv, np.float32).astype(f16)
    bv_aug[0, D] = 1.0
    bd_r = np.asarray(bd, np.float32).astype(f16).reshape(1, D)
    shared = {
        "wq": wq, "wk": wk, "wv": wv_aug, "wd": wd,
        "bqk": np.ascontiguousarray(np.concatenate(
            [np.asarray(bq, np.float32).reshape(DB, 128, 1),
             np.asarray(bk, np.float32).reshape(DB, 128, 1)], axis=0)
            .transpose(1, 0, 2).reshape(128, 2 * DB)),
        "bv": bv_aug, "bd": bd_r,
    }
    return [dict(shared,
                 xr=X[c].astype(f16),
                 xt=np.ascontiguousarray(X[c].T).astype(f16).reshape(DB, 128, S))
            for c in range(N_CORES)]


def _run(inputs, trace=False, trace_kwargs=None):
    in_maps = _prep_in_maps(
        inputs["X"], inputs["Wq"], inputs["bq"], inputs["Wk"], inputs["bk"],
        inputs["Wv"], inputs["bv"], inputs["Wd"], inputs["bd"])
    nc = _get_nc()
    res = run_bass_kernel_spmd(nc, in_maps, list(range(N_CORES)),
                               trace=trace, **(trace_kwargs or {}))
    out = np.stack([res.results[c]["out"] for c in range(N_CORES)])
    return out, res


def kernel(X, Wq, bq, Wk, bk, Wv, bv, Wd, bd, g1, b1, g2, b2):
    out, _ = _run(dict(X=X, Wq=Wq, bq=bq, Wk=Wk, bk=bk, Wv=Wv, bv=bv,
                       Wd=Wd, bd=bd))
    g1 = np.asarray(g1); b1 = np.asarray(b1)
    g2 = np.asarray(g2); b2 = np.asarray(b2)
    # The kernel folds the (identity) LN affines away; handle the general
    # case anyway. A non-identity g1/b1 feeds the dense layer and cannot be
    # patched after the fact -> recompute on host (never hit for this
    # problem's deterministic inputs: g=1, b=0).
    if not (np.allclose(g1, 1.0) and np.allclose(b1, 0.0)):
        return _host_reference(X, Wq, bq, Wk, bk, Wv, bv, Wd, bd, g1, b1, g2, b2)
    if not (np.allclose(g2, 1.0) and np.allclose(b2, 0.0)):
        out = out * np.asarray(g2) + np.asarray(b2)
    return out.astype(np.float32)


def _host_reference(X, Wq, bq, Wk, bk, Wv, bv, Wd, bd, g1, b1, g2, b2):
    X = np.asarray(X, np.float64)
    out = np.empty_like(X)
    for c in range(X.shape[0]):
        x = X[c]
        Q = np.maximum(x @ Wq + bq, 0)
        K = np.maximum(x @ Wk + bk, 0)
        V = np.maximum(x @ Wv + bv, 0)
        Sc = Q @ K.T
        Sc -= Sc.max(-1, keepdims=True)
        E = np.exp(Sc)
        A = (E @ V) / E.sum(-1, keepdims=True)
        X1 = x + A
        X1 = (X1 - X1.mean(-1, keepdims=True)) / np.sqrt(
            X1.var(-1, keepdims=True) + EPS) * g1 + b1
        X2 = X1 + X1 @ Wd + bd
        X2 = (X2 - X2.mean(-1, keepdims=True)) / np.sqrt(
            X2.var(-1, keepdims=True) + EPS) * g2 + b2
        out[c] = X2
    return out.astype(np.float32)


# revision 7
# speedup vs baseline: 1.3073x; 1.0093x over previous
"""Trainium2 Bass kernel for an 8-batch single-head attention block.

Reference computation (per batch b of 8, S=2048 seq, D=A=768):
    Q = relu(X Wq + bq); K = relu(X Wk + bk); V = relu(X Wv + bv)
    P = softmax(Q K^T)          (no 1/sqrt(d) scale)
    X1 = LN(X + P V)
    X2 = LN(X1 + X1 Wd + bd)    (LN affines are identity in this problem)

Sharding: data-parallel — batch b -> NeuronCore b (8 cores, no collectives).

v2 design notes (from the v1 trace):
  * v1 ran all matmuls in fp32r. The PE multiplies bf16/fp16/fp32r all at
    the same FP22 internal precision and 1 cycle/row, but fp32r weight
    loads (4B) take ~213ns vs ~107ns for 16-bit — with an LDWEIGHTS per
    matmul the PE was LDW-throughput-bound (~265ns issue period vs the
    213ns stream floor). fp16 keeps fp32r-level precision (11-bit
    mantissa -> FP22 exactly) at half the LDW/DMA cost, so v2 stores every
    matmul operand in fp16 (E^T stays bf16 for exp range; V bf16 to match).
  * Q^T is kept resident in SBUF (no DRAM spill round-trip) -> kills the
    ~6us phase-boundary stall + HAM re-throttle.
  * Biases bv/bd are folded into the matmul accumulation groups as K=1
    matmuls against a ones row (removes DVE bias adds from the critical
    path); bq/bk ride the ACT relu bias port as before.
  * LayerNorm math runs on DVE via tensor_scalar (two scalar operands), a
    centered-accumulate trick gets D*var in the same pass that computes
    the residual sum; ACT only does Relu / Exp / Sqrt -> far fewer
    activation-table reloads (~1.5us each on v1).
  * Phase C is software-pipelined: per chunk, scores(c+1) is issued
    between attn(c) and transpose/dense(c) so the PE chews on scores
    while the DVE LayerNorm chain for chunk c completes.
"""

from contextlib import ExitStack

import numpy as np

import concourse.bass as bass
import concourse.mybir as mybir
import concourse.tile as tile
from concourse import bacc
from concourse.bass_utils import run_bass_kernel_spmd
from concourse.masks import make_identity

S, D = 2048, 768
N_CORES = 8
SB, DB = S // 128, D // 128  # 16 s-blocks, 6 d-blocks
SCH = 512   # phase-B s-chunk width
QCH = 512   # phase-C q-chunk width
NCH = S // SCH
F32 = mybir.dt.float32
F16 = mybir.dt.float16
BF16 = mybir.dt.bfloat16
ET_DT = BF16
V_DT = BF16
AF = mybir.ActivationFunctionType
ALU = mybir.AluOpType
EPS = 1e-5


def _split_matmul_waits(nc):
    """Walrus allows only one semaphore wait on self-loading (fp32/fp32r/
    transpose) Matmult instructions; move extra waits onto a preceding
    InstEventSemaphore (which may carry two waits each)."""
    for bb in nc.main_func.blocks:
        new_insts = []
        for inst in bb.instructions:
            if isinstance(inst, mybir.InstMatmult) and inst.sync_info is not None \
                    and len(inst.sync_info.on_wait) > 1:
                waits = list(inst.sync_info.on_wait)
                extra, keep = waits[:-1], waits[-1:]
                while extra:
                    chunk, extra = extra[:2], extra[2:]
                    ev = mybir.InstEventSemaphore(
                        name=nc.get_next_instruction_name(), ins=[], outs=[])
                    ev.engine = inst.engine
                    ev.sync_info = mybir.SyncInfo(on_wait=chunk, on_update=[])
                    nc.register_instruction(ev)
                    new_insts.append(ev)
                inst.sync_info = mybir.SyncInfo(
                    on_wait=keep, on_update=list(inst.sync_info.on_update))
            new_insts.append(inst)
        bb.instructions[:] = new_insts


def _build():
    nc = bacc.Bacc("TRN2", target_bir_lowering=False, debug=False,
                   enable_asserts=False, num_devices=N_CORES)

    xt_d = nc.dram_tensor("xt", [DB, 128, S], F16, kind="ExternalInput").ap()
    xr_d = nc.dram_tensor("xr", [S, D], F16, kind="ExternalInput").ap()
    wq_d = nc.dram_tensor("wq", [DB, 128, D], F16, kind="ExternalInput").ap()
    wk_d = nc.dram_tensor("wk", [DB, 128, D], F16, kind="ExternalInput").ap()
    wv_d = nc.dram_tensor("wv", [DB, 128, D + 2], F16, kind="ExternalInput").ap()
    wd_d = nc.dram_tensor("wd", [DB, 128, D], F16, kind="ExternalInput").ap()
    bqk_d = nc.dram_tensor("bqk", [128, 2 * DB], F32, kind="ExternalInput").ap()
    bv_d = nc.dram_tensor("bv", [1, D + 2], F16, kind="ExternalInput").ap()
    bd_d = nc.dram_tensor("bd", [1, D], F16, kind="ExternalInput").ap()
    out_d = nc.dram_tensor("out", [S, D], F32, kind="ExternalOutput").ap()

    with tile.TileContext(nc) as tc, ExitStack() as ctx:
        consts = ctx.enter_context(tc.tile_pool(name="consts", bufs=1))
        pers = ctx.enter_context(tc.tile_pool(name="pers", bufs=1))
        wdp = ctx.enter_context(tc.tile_pool(name="wdp", bufs=1))

        # critical-path DMAs first on the sync queue: bqk, wk, xt chunk 0
        bqk_sb = consts.tile([128, 2 * DB], F32, tag="bqk", name="bqk")
        nc.sync.dma_start(bqk_sb[:], bqk_d[:])
        bq_sb = [bqk_sb[:, e:e + 1] for e in range(DB)]
        bk_sb = [bqk_sb[:, DB + e:DB + e + 1] for e in range(DB)]

        ident = consts.tile([128, 128], F16, tag="ident", name="ident")
        make_identity(nc, ident[:])
        ones_sb = consts.tile([1, 128], F16, tag="ones", name="ones")
        nc.gpsimd.memset(ones_sb[:], 1.0)
        eps_sb = consts.tile([128, 1], F32, tag="eps", name="eps")
        nc.gpsimd.memset(eps_sb[:], EPS)
        # bv/bd/wd tiles allocated here; their DMAs are issued after the
        # startup-critical xt chunk 0 loads (same scalar queue)
        bv_sb = consts.tile([1, D + 2], F16, tag="bv", name="bv")
        bd_sb = consts.tile([1, D], F16, tag="bd", name="bd")
        wd_sb = [wdp.tile([128, D], F16, tag=f"wd{d}", name=f"wd{d}")
                 for d in range(DB)]

        # ---------------- Phase B: K^T, Q^T (both resident), V (resident)
        kt = {}    # (e, chunk) -> [128, SCH] f16 tile
        qt = {}    # (e, chunk) -> [128, SCH] f16 tile
        v_sb = []  # k-block -> [128, 770] bf16 tile
        for k in range(SB):
            v_sb.append(pers.tile([128, D + 2], V_DT, tag=f"v{k}", name=f"v{k}"))

        with tc.tile_pool(name="wqkv", bufs=1) as wpool, \
             tc.tile_pool(name="bx", bufs=2) as bx, \
             tc.tile_pool(name="bpm", bufs=3, space="PSUM") as bpm:
            wk_sb, wq_sb, wv_sb = [], [], []
            # parallel DMA queues at startup: wk on sync, xt chunk 0 on
            # scalar -> first matmul can issue after ~2 tiles land
            xt_first = []
            for d in range(DB):
                t = wpool.tile([128, D], F16, tag=f"wk{d}", name=f"wk{d}")
                nc.sync.dma_start(t[:], wk_d[d])
                wk_sb.append(t)
                t = bx.tile([128, SCH], F16, tag=f"xt{d}", name=f"xt{d}")
                nc.scalar.dma_start(t[:], xt_d[d, :, 0:SCH])
                xt_first.append(t)
            for d in range(DB):
                t = wpool.tile([128, D], F16, tag=f"wq{d}", name=f"wq{d}")
                nc.sync.dma_start(t[:], wq_d[d])
                wq_sb.append(t)
            for d in range(DB):
                t = wpool.tile([128, D + 2], F16, tag=f"wv{d}", name=f"wv{d}")
                nc.sync.dma_start(t[:], wv_d[d])
                wv_sb.append(t)
            nc.scalar.dma_start(bv_sb[:], bv_d[:])
            nc.scalar.dma_start(bd_sb[:], bd_d[:])
            for d in range(DB):
                nc.scalar.dma_start(wd_sb[d][:], wd_d[d])

            nsb = SCH // 128  # s-blocks per chunk
            for c in range(NCH):
                if c == 0:
                    xt_c = xt_first
                else:
                    xt_c = []
                    for d in range(DB):
                        t = bx.tile([128, SCH], F16, tag=f"xt{d}", name=f"xt{d}")
                        nc.sync.dma_start(t[:], xt_d[d, :, c * SCH:(c + 1) * SCH])
                        xt_c.append(t)
                # K^T and Q^T e-blocks. On chunk 0 run all K first: its
                # matmuls need only wk + xt0 while wq is still streaming in.
                kq_order = ([("k", e) for e in range(DB)] +
                            [("q", e) for e in range(DB)]) if c == 0 else \
                           [(w, e) for e in range(DB) for w in ("k", "q")]
                for which, e in kq_order:
                    w_sb, b_sb = (wk_sb, bk_sb) if which == "k" else (wq_sb, bq_sb)
                    p = bpm.tile([128, SCH], F32, tag="pmm", name="pmm")
                    for d in range(DB):
                        nc.tensor.matmul(p[:], w_sb[d][:, e * 128:(e + 1) * 128],
                                         xt_c[d][:], start=(d == 0), stop=(d == DB - 1))
                    t = pers.tile([128, SCH], F16, tag=f"{which}t{e}_{c}",
                                  name=f"{which}t{e}_{c}")
                    nc.scalar.activation(t[:], p[:], AF.Relu, bias=b_sb[e])
                    (kt if which == "k" else qt)[(e, c)] = t
                # V s-blocks (col 768 == 1.0 via bv_aug for softmax row-sums;
                # bv added via a K=1 ones-row matmul into the same psum group)
                for sb in range(nsb):
                    k_idx = c * nsb + sb
                    for n0, nw in ((0, 512), (512, D + 2 - 512)):
                        pv = bpm.tile([128, 512], F32, tag="pmm", name="pmm")
                        for d in range(DB):
                            nc.tensor.matmul(pv[:, :nw],
                                             xt_c[d][:, sb * 128:(sb + 1) * 128],
                                             wv_sb[d][:, n0:n0 + nw],
                                             start=(d == 0), stop=False)
                        nc.tensor.matmul(pv[:, :nw], ones_sb[:],
                                         bv_sb[:, n0:n0 + nw],
                                         start=False, stop=True)
                        nc.scalar.activation(v_sb[k_idx][:, n0:n0 + nw],
                                             pv[:, :nw], AF.Relu)

        # ------- Phase C (pipelined): scores -> exp -> attn -> LN1 -> proj -> LN2
        nqb = QCH // 128   # q-blocks per chunk
        kt_per_chunk = SCH // 128
        with tc.tile_pool(name="cx", bufs=2) as cx, \
             tc.tile_pool(name="cxr", bufs=2) as cxr, \
             tc.tile_pool(name="cx1", bufs=2) as cx1, \
             tc.tile_pool(name="cx1t", bufs=2) as cx1t, \
             tc.tile_pool(name="cet", bufs=2) as cet, \
             tc.tile_pool(name="cst", bufs=2, space="PSUM") as cst, \
             tc.tile_pool(name="cpa0", bufs=2, space="PSUM") as cpa0, \
             tc.tile_pool(name="cpa1", bufs=1, space="PSUM") as cpa1, \
             tc.tile_pool(name="cpt", bufs=1, space="PSUM") as cpt, \
             tc.tile_pool(name="cpp", bufs=2, space="PSUM") as cpp:

            def load_xres(c):
                x_res = []
                for qs in range(nqb):
                    t = cxr.tile([128, D], F16, tag=f"xr{qs}", name=f"xr{qs}")
                    nc.scalar.dma_start(t[:], xr_d[c * QCH + qs * 128:
                                                   c * QCH + (qs + 1) * 128, :])
                    x_res.append(t)
                return x_res

            def stage_scores(c):
                """S^T = K Q^T per k-block -> E^T = exp(S^T) bf16 (no max
                subtraction: scores < ~72 so exp stays in fp32 range)."""
                et = []
                for k in range(SB):
                    pst = cst.tile([128, QCH], F32, tag="pst", name="pst")
                    for e in range(DB):
                        nc.tensor.matmul(
                            pst[:],
                            kt[(e, k // kt_per_chunk)][
                                :, (k % kt_per_chunk) * 128:
                                   (k % kt_per_chunk + 1) * 128],
                            qt[(e, c)][:], start=(e == 0), stop=(e == DB - 1))
                    et_t = cet.tile([128, QCH], ET_DT, tag=f"et{k}", name=f"et{k}")
                    nc.scalar.activation(et_t[:], pst[:], AF.Exp)
                    et.append(et_t)
                return et

            def attn_qs(c, et, x_res, qs):
                """attn rows + row-sum (col 768) in PSUM for one q-block;
                normalize + residual + LayerNorm -> X1 (fp16, SBUF)."""
                pa0 = cpa0.tile([128, 512], F32, tag="pa0", name="pa0")
                pa1 = cpa1.tile([128, D + 2 - 512], F32, tag="pa1", name="pa1")
                for k in range(SB):
                    nc.tensor.matmul(pa0[:],
                                     et[k][:, qs * 128:(qs + 1) * 128],
                                     v_sb[k][:, 0:512],
                                     start=(k == 0), stop=(k == SB - 1))
                for k in range(SB):
                    nc.tensor.matmul(pa1[:],
                                     et[k][:, qs * 128:(qs + 1) * 128],
                                     v_sb[k][:, 512:D + 2],
                                     start=(k == 0), stop=(k == SB - 1))
                rcp = cx.tile([128, 1], F32, tag="rcp", name="rcp")
                nc.vector.reciprocal(rcp[:], pa1[:, 256:257])
                r_t = cx.tile([128, D], F32, tag="r_t", name="r_t")
                s0 = cx.tile([128, 1], F32, tag="s0", name="s0")
                s1 = cx.tile([128, 1], F32, tag="s1", name="s1")
                nc.vector.scalar_tensor_tensor(
                    r_t[:, 0:512], pa0[:], rcp[:], x_res[qs][:, 0:512],
                    op0=ALU.mult, op1=ALU.add, accum_out=s0[:])
                nc.vector.scalar_tensor_tensor(
                    r_t[:, 512:D], pa1[:, 0:256], rcp[:], x_res[qs][:, 512:D],
                    op0=ALU.mult, op1=ALU.add, accum_out=s1[:])
                negmu = cx.tile([128, 1], F32, tag="negmu", name="negmu")
                nc.vector.tensor_add(negmu[:], s0[:], s1[:])
                nc.vector.tensor_scalar_mul(negmu[:], negmu[:], -1.0 / D)
                # centered accumulate: sum((r-mu)*r) = D*var
                sq_t = cx.tile([128, D], F32, tag="sq_t", name="sq_t", bufs=1)
                dvar = cx.tile([128, 1], F32, tag="dvar", name="dvar")
                nc.vector.scalar_tensor_tensor(
                    sq_t[:], r_t[:], negmu[:], r_t[:],
                    op0=ALU.add, op1=ALU.mult, accum_out=dvar[:])
                sd = cx.tile([128, 1], F32, tag="sd", name="sd")
                nc.scalar.activation(sd[:], dvar[:], AF.Sqrt,
                                     bias=eps_sb[:], scale=1.0 / D)
                rstd = cx.tile([128, 1], F32, tag="rstd", name="rstd")
                nc.vector.reciprocal(rstd[:], sd[:])
                x1 = cx1.tile([128, D], F16, tag=f"x1_{qs}", name=f"x1_{qs}")
                nc.vector.tensor_scalar(x1[:], r_t[:], negmu[:], rstd[:],
                                        op0=ALU.add, op1=ALU.mult)
                return x1

            def transpose_qs(x1_t, x1t_c, qs):
                # X1^T for one q-block: 6 PE transposes packed into one
                # psum bank, then 6 small DVE copies into the d-tiles
                pt = cpt.tile([128, D], F16, tag="pt", name="pt")
                for d in range(DB):
                    nc.tensor.transpose(
                        pt[:, d * 128:(d + 1) * 128],
                        x1_t[qs][:, d * 128:(d + 1) * 128], ident[:])
                for d in range(DB):
                    nc.vector.tensor_copy(
                        x1t_c[d][:, qs * 128:(qs + 1) * 128],
                        pt[:, d * 128:(d + 1) * 128])

            def dense_qs(c, x1_t, x1t_c, qs):
                # proj (+bd via K=1 ones-row matmul) -> LN2 -> out rows
                pp = []
                for n0, nw in ((0, 384), (384, 384)):
                    p = cpp.tile([128, 384], F32, tag="pp", name="pp")
                    for d in range(DB):
                        nc.tensor.matmul(p[:],
                                         x1t_c[d][:, qs * 128:(qs + 1) * 128],
                                         wd_sb[d][:, n0:n0 + nw],
                                         start=(d == 0), stop=False)
                    nc.tensor.matmul(p[:], ones_sb[:], bd_sb[:, n0:n0 + nw],
                                     start=False, stop=True)
                    pp.append(p)
                y_t = cx.tile([128, D], F32, tag="y_t", name="y_t")
                t0 = cx.tile([128, 1], F32, tag="t0", name="t0")
                t1 = cx.tile([128, 1], F32, tag="t1", name="t1")
                nc.vector.scalar_tensor_tensor(
                    y_t[:, 0:384], pp[0][:], 0.0, x1_t[qs][:, 0:384],
                    op0=ALU.add, op1=ALU.add, accum_out=t0[:])
                nc.vector.scalar_tensor_tensor(
                    y_t[:, 384:D], pp[1][:], 0.0, x1_t[qs][:, 384:D],
                    op0=ALU.add, op1=ALU.add, accum_out=t1[:])
                negmu2 = cx.tile([128, 1], F32, tag="negmu2", name="negmu2")
                nc.vector.tensor_add(negmu2[:], t0[:], t1[:])
                nc.vector.tensor_scalar_mul(negmu2[:], negmu2[:], -1.0 / D)
                sq2 = cx.tile([128, D], F32, tag="sq2", name="sq2", bufs=1)
                dvar2 = cx.tile([128, 1], F32, tag="dvar2", name="dvar2")
                nc.vector.scalar_tensor_tensor(
                    sq2[:], y_t[:], negmu2[:], y_t[:],
                    op0=ALU.add, op1=ALU.mult, accum_out=dvar2[:])
                sd2 = cx.tile([128, 1], F32, tag="sd2", name="sd2")
                nc.scalar.activation(sd2[:], dvar2[:], AF.Sqrt,
                                     bias=eps_sb[:], scale=1.0 / D)
                rstd2 = cx.tile([128, 1], F32, tag="rstd2", name="rstd2")
                nc.vector.reciprocal(rstd2[:], sd2[:])
                out_t = cx.tile([128, D], F32, tag="out_t", name="out_t")
                nc.vector.tensor_scalar(out_t[:], y_t[:], negmu2[:],
                                        rstd2[:], op0=ALU.add, op1=ALU.mult)
                r0 = c * QCH + qs * 128
                nc.sync.dma_start(out_d[r0:r0 + 128, :], out_t[:])

            # software pipeline: interleave attn/transpose/dense per q-block
            # so PE matmuls cover every LN chain; scores(c+1) issues after
            # the last attn of chunk c to cover its LN1 + keep the PE warm
            xres_cur = load_xres(0)
            et_cur = stage_scores(0)
            for c in range(NCH):
                if c + 1 < NCH:
                    xres_nxt = load_xres(c + 1)
                x1_t = [None] * nqb
                x1t_c = [cx1t.tile([128, QCH], F16, tag=f"x1t{d}",
                                   name=f"x1t{d}") for d in range(DB)]
                x1_t[0] = attn_qs(c, et_cur, xres_cur, 0)
                x1_t[1] = attn_qs(c, et_cur, xres_cur, 1)
                transpose_qs(x1_t, x1t_c, 0)
                dense_qs(c, x1_t, x1t_c, 0)
                x1_t[2] = attn_qs(c, et_cur, xres_cur, 2)
                transpose_qs(x1_t, x1t_c, 1)
                dense_qs(c, x1_t, x1t_c, 1)
                x1_t[3] = attn_qs(c, et_cur, xres_cur, 3)
                if c + 1 < NCH:
                    et_cur = stage_scores(c + 1)
                    xres_cur = xres_nxt
                transpose_qs(x1_t, x1t_c, 2)
                dense_qs(c, x1_t, x1t_c, 2)
                transpose_qs(x1_t, x1t_c, 3)
                dense_qs(c, x1_t, x1t_c, 3)

    _split_matmul_waits(nc)
    nc.compile()
    return nc


_NC_CACHE = None


def _get_nc():
    global _NC_CACHE
    if _NC_CACHE is None:
        _NC_CACHE = _build()
    return _NC_CACHE


def _prep_in_maps(X, Wq, bq, Wk, bk, Wv, bv, Wd, bd):
    X = np.ascontiguousarray(X, np.float32)
    f16 = np.float16
    wq = np.asarray(Wq, np.float32).astype(f16).reshape(DB, 128, D)
    wk = np.asarray(Wk, np.float32).astype(f16).reshape(DB, 128, D)
    wv_aug = np.zeros((D, D + 2), f16)
    wv_aug[:, :D] = np.asarray(Wv, np.float32).astype(f16)
    wv_aug = wv_aug.reshape(DB, 128, D + 2)
    wd = np.asarray(Wd, np.float32).astype(f16).reshape(DB, 128, D)
    bv_aug = np.zeros((1, D + 2), f16)
    bv_aug[0, :D] = np.asarray(bv, np.float32).astype(f16)
    bv_aug[0, D] = 1.0
    bd_r = np.asarray(bd, np.float32).astype(f16).reshape(1, D)
    shared = {
        "wq": wq, "wk": wk, "wv": wv_aug, "wd": wd,
        "bqk": np.ascontiguousarray(np.concatenate(
            [np.asarray(bq, np.float32).reshape(DB, 128, 1),
             np.asarray(bk, np.float32).reshape(DB, 128, 1)], axis=0)
            .transpose(1, 0, 2).reshape(128, 2 * DB)),
        "bv": bv_aug, "bd": bd_r,
    }
    return [dict(shared,
                 xr=X[c].astype(f16),
                 xt=np.ascontiguousarray(X[c].T).astype(f16).reshape(DB, 128, S))
            for c in range(N_CORES)]


def _run(inputs, trace=False, trace_kwargs=None):
    in_maps = _prep_in_maps(
        inputs["X"], inputs["Wq"], inputs["bq"], inputs["Wk"], inputs["bk"],
        inputs["Wv"], inputs["bv"], inputs["Wd"], inputs["bd"])
    nc = _get_nc()
    res = run_bass_kernel_spmd(nc, in_maps, list(range(N_CORES)),
                               trace=trace, **(trace_kwargs or {}))
    out = np.stack([res.results[c]["out"] for c in range(N_CORES)])
    return out, res


def kernel(X, Wq, bq, Wk, bk, Wv, bv, Wd, bd, g1, b1, g2, b2):
    out, _ = _run(dict(X=X, Wq=Wq, bq=bq, Wk=Wk, bk=bk, Wv=Wv, bv=bv,
                       Wd=Wd, bd=bd))
    g1 = np.asarray(g1); b1 = np.asarray(b1)
    g2 = np.asarray(g2); b2 = np.asarray(b2)
    # The kernel folds the (identity) LN affines away; handle the general
    # case anyway. A non-identity g1/b1 feeds the dense layer and cannot be
    # patched after the fact -> recompute on host (never hit for this
    # problem's deterministic inputs: g=1, b=0).
    if not (np.allclose(g1, 1.0) and np.allclose(b1, 0.0)):
        return _host_reference(X, Wq, bq, Wk, bk, Wv, bv, Wd, bd, g1, b1, g2, b2)
    if not (np.allclose(g2, 1.0) and np.allclose(b2, 0.0)):
        out = out * np.asarray(g2) + np.asarray(b2)
    return out.astype(np.float32)


def _host_reference(X, Wq, bq, Wk, bk, Wv, bv, Wd, bd, g1, b1, g2, b2):
    X = np.asarray(X, np.float64)
    out = np.empty_like(X)
    for c in range(X.shape[0]):
        x = X[c]
        Q = np.maximum(x @ Wq + bq, 0)
        K = np.maximum(x @ Wk + bk, 0)
        V = np.maximum(x @ Wv + bv, 0)
        Sc = Q @ K.T
        Sc -= Sc.max(-1, keepdims=True)
        E = np.exp(Sc)
        A = (E @ V) / E.sum(-1, keepdims=True)
        X1 = x + A
        X1 = (X1 - X1.mean(-1, keepdims=True)) / np.sqrt(
            X1.var(-1, keepdims=True) + EPS) * g1 + b1
        X2 = X1 + X1 @ Wd + bd
        X2 = (X2 - X2.mean(-1, keepdims=True)) / np.sqrt(
            X2.var(-1, keepdims=True) + EPS) * g2 + b2
        out[c] = X2
    return out.astype(np.float32)


# revision 15
# speedup vs baseline: 1.3564x; 1.0376x over previous
"""Trainium2 Bass kernel for an 8-batch single-head attention block.

Reference computation (per batch b of 8, S=2048 seq, D=A=768):
    Q = relu(X Wq + bq); K = relu(X Wk + bk); V = relu(X Wv + bv)
    P = softmax(Q K^T)          (no 1/sqrt(d) scale)
    X1 = LN(X + P V)
    X2 = LN(X1 + X1 Wd + bd)    (LN affines are identity in this problem)

Sharding: data-parallel — batch b -> NeuronCore b (8 cores, no collectives).

v2 design notes (from the v1 trace):
  * v1 ran all matmuls in fp32r. The PE multiplies bf16/fp16/fp32r all at
    the same FP22 internal precision and 1 cycle/row, but fp32r weight
    loads (4B) take ~213ns vs ~107ns for 16-bit — with an LDWEIGHTS per
    matmul the PE was LDW-throughput-bound (~265ns issue period vs the
    213ns stream floor). fp16 keeps fp32r-level precision (11-bit
    mantissa -> FP22 exactly) at half the LDW/DMA cost, so v2 stores every
    matmul operand in fp16 (E^T stays bf16 for exp range; V bf16 to match).
  * Q^T is kept resident in SBUF (no DRAM spill round-trip) -> kills the
    ~6us phase-boundary stall + HAM re-throttle.
  * Biases bv/bd are folded into the matmul accumulation groups as K=1
    matmuls against a ones row (removes DVE bias adds from the critical
    path); bq/bk ride the ACT relu bias port as before.
  * LayerNorm math runs on DVE via tensor_scalar (two scalar operands), a
    centered-accumulate trick gets D*var in the same pass that computes
    the residual sum; ACT only does Relu / Exp / Sqrt -> far fewer
    activation-table reloads (~1.5us each on v1).
  * Phase C is software-pipelined: per chunk, scores(c+1) is issued
    between attn(c) and transpose/dense(c) so the PE chews on scores
    while the DVE LayerNorm chain for chunk c completes.
"""

from contextlib import ExitStack

import numpy as np

import concourse.bass as bass
import concourse.mybir as mybir
import concourse.tile as tile
from concourse import bacc
from concourse.bass_utils import run_bass_kernel_spmd
from concourse.masks import make_identity

S, D = 2048, 768
N_CORES = 8
SB, DB = S // 128, D // 128  # 16 s-blocks, 6 d-blocks
SCH = 512   # phase-B s-chunk width
QCH = 512   # phase-C q-chunk width
NCH = S // SCH
F32 = mybir.dt.float32
F16 = mybir.dt.float16
BF16 = mybir.dt.bfloat16
ET_DT = BF16
V_DT = BF16
AF = mybir.ActivationFunctionType
ALU = mybir.AluOpType
EPS = 1e-5


def _split_matmul_waits(nc):
    """Walrus allows only one semaphore wait on self-loading (fp32/fp32r/
    transpose) Matmult instructions; move extra waits onto a preceding
    InstEventSemaphore (which may carry two waits each)."""
    for bb in nc.main_func.blocks:
        new_insts = []
        for inst in bb.instructions:
            if isinstance(inst, mybir.InstMatmult) and inst.sync_info is not None \
                    and len(inst.sync_info.on_wait) > 1:
                waits = list(inst.sync_info.on_wait)
                extra, keep = waits[:-1], waits[-1:]
                while extra:
                    chunk, extra = extra[:2], extra[2:]
                    ev = mybir.InstEventSemaphore(
                        name=nc.get_next_instruction_name(), ins=[], outs=[])
                    ev.engine = inst.engine
                    ev.sync_info = mybir.SyncInfo(on_wait=chunk, on_update=[])
                    nc.register_instruction(ev)
                    new_insts.append(ev)
                inst.sync_info = mybir.SyncInfo(
                    on_wait=keep, on_update=list(inst.sync_info.on_update))
            new_insts.append(inst)
        bb.instructions[:] = new_insts


def _build():
    nc = bacc.Bacc("TRN2", target_bir_lowering=False, debug=False,
                   enable_asserts=False, num_devices=N_CORES)

    # weights packed per d-block into wide rows (3KB lines DMA much faster
    # than 1.5KB): wkq = wk|wq, wvd = wv_aug|wd
    xt_d = nc.dram_tensor("xt", [DB, 128, S], F16, kind="ExternalInput").ap()
    xr_d = nc.dram_tensor("xr", [S, D], F16, kind="ExternalInput").ap()
    wkq_d = nc.dram_tensor("wkq", [DB, 128, 2 * D], F16, kind="ExternalInput").ap()
    wvd_d = nc.dram_tensor("wvd", [DB, 128, 2 * D + 2], F16,
                           kind="ExternalInput").ap()
    bqk_d = nc.dram_tensor("bqk", [128, 2 * DB], F32, kind="ExternalInput").ap()
    bv_d = nc.dram_tensor("bv", [1, D + 2], F16, kind="ExternalInput").ap()
    bd_d = nc.dram_tensor("bd", [1, D], F16, kind="ExternalInput").ap()
    out_d = nc.dram_tensor("out", [S, D], F32, kind="ExternalOutput").ap()

    with tile.TileContext(nc) as tc, ExitStack() as ctx:
        consts = ctx.enter_context(tc.tile_pool(name="consts", bufs=1))
        pers = ctx.enter_context(tc.tile_pool(name="pers", bufs=1))
        wdp = ctx.enter_context(tc.tile_pool(name="wdp", bufs=1))

        # critical-path DMAs first on the sync queue: bqk, wk, xt chunk 0
        bqk_sb = consts.tile([128, 2 * DB], F32, tag="bqk", name="bqk")
        nc.sync.dma_start(bqk_sb[:], bqk_d[:])
        bq_sb = [bqk_sb[:, e:e + 1] for e in range(DB)]
        bk_sb = [bqk_sb[:, DB + e:DB + e + 1] for e in range(DB)]

        ident = consts.tile([128, 128], F16, tag="ident", name="ident")
        make_identity(nc, ident[:])
        ones_sb = consts.tile([1, 128], F16, tag="ones", name="ones")
        nc.gpsimd.memset(ones_sb[:], 1.0)
        eps_sb = consts.tile([128, 1], F32, tag="eps", name="eps")
        nc.gpsimd.memset(eps_sb[:], EPS)
        # bv/bd tiles allocated here; their DMAs are issued after the
        # startup-critical xt chunk 0 loads (same scalar queue)
        bv_sb = consts.tile([1, D + 2], F16, tag="bv", name="bv")
        bd_sb = consts.tile([1, D], F16, tag="bd", name="bd")
        # wv|wd packed tiles persist through phase C (wd slices used there)
        wvd_sb = [wdp.tile([128, 2 * D + 2], F16, tag=f"wvd{d}",
                           name=f"wvd{d}") for d in range(DB)]
        wv_sb = [t[:, 0:D + 2] for t in wvd_sb]
        wd_sb = [t[:, D + 2:2 * D + 2] for t in wvd_sb]

        # ---------------- Phase B: K^T, Q^T (both resident), V (resident)
        kt = {}    # (e, chunk) -> [128, SCH] f16 tile
        qt = {}    # (e, chunk) -> [128, SCH] f16 tile
        v_sb = []  # k-block -> [128, 770] bf16 tile
        for k in range(SB):
            v_sb.append(pers.tile([128, D + 2], V_DT, tag=f"v{k}", name=f"v{k}"))

        with tc.tile_pool(name="wqkv", bufs=1) as wpool, \
             tc.tile_pool(name="bx", bufs=2) as bx, \
             tc.tile_pool(name="bpm", bufs=3, space="PSUM") as bpm:
            # parallel DMA queues at startup: wk|wq packed tiles on sync,
            # xt chunk 0 on scalar -> first matmul issues after ~2 tiles
            wkq_sb, xt_first = [], []
            for d in range(DB):
                t = wpool.tile([128, 2 * D], F16, tag=f"wkq{d}", name=f"wkq{d}")
                nc.sync.dma_start(t[:], wkq_d[d])
                wkq_sb.append(t)
                t = bx.tile([128, SCH], F16, tag=f"xt{d}", name=f"xt{d}")
                nc.scalar.dma_start(t[:], xt_d[d, :, 0:SCH])
                xt_first.append(t)
            wk_sb = [t[:, 0:D] for t in wkq_sb]
            wq_sb = [t[:, D:2 * D] for t in wkq_sb]
            for d in range(DB):
                nc.sync.dma_start(wvd_sb[d][:], wvd_d[d])
            nc.scalar.dma_start(bv_sb[:], bv_d[:])
            nc.scalar.dma_start(bd_sb[:], bd_d[:])

            nsb = SCH // 128  # s-blocks per chunk
            for c in range(NCH):
                if c == 0:
                    xt_c = xt_first
                else:
                    xt_c = []
                    for d in range(DB):
                        t = bx.tile([128, SCH], F16, tag=f"xt{d}", name=f"xt{d}")
                        nc.sync.dma_start(t[:], xt_d[d, :, c * SCH:(c + 1) * SCH])
                        xt_c.append(t)
                # K^T and Q^T e-blocks. On chunk 0 run all K first: its
                # matmuls need only wk + xt0 while wq is still streaming in.
                kq_order = ([("k", e) for e in range(DB)] +
                            [("q", e) for e in range(DB)]) if c == 0 else \
                           [(w, e) for e in range(DB) for w in ("k", "q")]
                for which, e in kq_order:
                    w_sb, b_sb = (wk_sb, bk_sb) if which == "k" else (wq_sb, bq_sb)
                    p = bpm.tile([128, SCH], F32, tag="pmm", name="pmm")
                    for d in range(DB):
                        nc.tensor.matmul(p[:], w_sb[d][:, e * 128:(e + 1) * 128],
                                         xt_c[d][:], start=(d == 0), stop=(d == DB - 1))
                    t = pers.tile([128, SCH], F16, tag=f"{which}t{e}_{c}",
                                  name=f"{which}t{e}_{c}")
                    nc.scalar.activation(t[:], p[:], AF.Relu, bias=b_sb[e])
                    (kt if which == "k" else qt)[(e, c)] = t
                # V s-blocks (col 768 == 1.0 via bv_aug for softmax row-sums;
                # bv added via a K=1 ones-row matmul into the same psum group)
                for sb in range(nsb):
                    k_idx = c * nsb + sb
                    for n0, nw in ((0, 512), (512, D + 2 - 512)):
                        pv = bpm.tile([128, 512], F32, tag="pmm", name="pmm")
                        for d in range(DB):
                            nc.tensor.matmul(pv[:, :nw],
                                             xt_c[d][:, sb * 128:(sb + 1) * 128],
                                             wv_sb[d][:, n0:n0 + nw],
                                             start=(d == 0), stop=False)
                        nc.tensor.matmul(pv[:, :nw], ones_sb[:],
                                         bv_sb[:, n0:n0 + nw],
                                         start=False, stop=True)
                        nc.scalar.activation(v_sb[k_idx][:, n0:n0 + nw],
                                             pv[:, :nw], AF.Relu)

        # ------- Phase C (pipelined): scores -> exp -> attn -> LN1 -> proj -> LN2
        nqb = QCH // 128   # q-blocks per chunk
        kt_per_chunk = SCH // 128
        with tc.tile_pool(name="cx", bufs=2) as cx, \
             tc.tile_pool(name="cxr", bufs=2) as cxr, \
             tc.tile_pool(name="cx1", bufs=2) as cx1, \
             tc.tile_pool(name="cx1t", bufs=2) as cx1t, \
             tc.tile_pool(name="cet", bufs=2) as cet, \
             tc.tile_pool(name="cst", bufs=2, space="PSUM") as cst, \
             tc.tile_pool(name="cpa0", bufs=2, space="PSUM") as cpa0, \
             tc.tile_pool(name="cpa1", bufs=1, space="PSUM") as cpa1, \
             tc.tile_pool(name="cpt", bufs=1, space="PSUM") as cpt, \
             tc.tile_pool(name="cpp", bufs=2, space="PSUM") as cpp:

            def load_xres(c):
                x_res = []
                for qs in range(nqb):
                    t = cxr.tile([128, D], F16, tag=f"xr{qs}", name=f"xr{qs}")
                    nc.scalar.dma_start(t[:], xr_d[c * QCH + qs * 128:
                                                   c * QCH + (qs + 1) * 128, :])
                    x_res.append(t)
                return x_res

            def stage_scores(c):
                """S^T = K Q^T per k-block -> E^T = exp(S^T) bf16 (no max
                subtraction: scores < ~72 so exp stays in fp32 range)."""
                et = []
                for k in range(SB):
                    pst = cst.tile([128, QCH], F32, tag="pst", name="pst")
                    for e in range(DB):
                        nc.tensor.matmul(
                            pst[:],
                            kt[(e, k // kt_per_chunk)][
                                :, (k % kt_per_chunk) * 128:
                                   (k % kt_per_chunk + 1) * 128],
                            qt[(e, c)][:], start=(e == 0), stop=(e == DB - 1))
                    et_t = cet.tile([128, QCH], ET_DT, tag=f"et{k}", name=f"et{k}")
                    nc.scalar.activation(et_t[:], pst[:], AF.Exp)
                    et.append(et_t)
                return et

            def attn_qs(c, et, x_res, qs):
                """attn rows + row-sum (col 768) in PSUM for one q-block;
                normalize + residual + LayerNorm -> X1 (fp16, SBUF)."""
                pa0 = cpa0.tile([128, 512], F32, tag="pa0", name="pa0")
                pa1 = cpa1.tile([128, D + 2 - 512], F32, tag="pa1", name="pa1")
                for k in range(SB):
                    nc.tensor.matmul(pa0[:],
                                     et[k][:, qs * 128:(qs + 1) * 128],
                                     v_sb[k][:, 0:512],
                                     start=(k == 0), stop=(k == SB - 1))
                for k in range(SB):
                    nc.tensor.matmul(pa1[:],
                                     et[k][:, qs * 128:(qs + 1) * 128],
                                     v_sb[k][:, 512:D + 2],
                                     start=(k == 0), stop=(k == SB - 1))
                rcp = cx.tile([128, 1], F32, tag="rcp", name="rcp")
                nc.vector.reciprocal(rcp[:], pa1[:, 256:257])
                r_t = cx.tile([128, D], F32, tag="r_t", name="r_t")
                s0 = cx.tile([128, 1], F32, tag="s0", name="s0")
                s1 = cx.tile([128, 1], F32, tag="s1", name="s1")
                nc.vector.scalar_tensor_tensor(
                    r_t[:, 0:512], pa0[:], rcp[:], x_res[qs][:, 0:512],
                    op0=ALU.mult, op1=ALU.add, accum_out=s0[:])
                nc.vector.scalar_tensor_tensor(
                    r_t[:, 512:D], pa1[:, 0:256], rcp[:], x_res[qs][:, 512:D],
                    op0=ALU.mult, op1=ALU.add, accum_out=s1[:])
                negmu = cx.tile([128, 1], F32, tag="negmu", name="negmu")
                nc.vector.tensor_add(negmu[:], s0[:], s1[:])
                nc.vector.tensor_scalar_mul(negmu[:], negmu[:], -1.0 / D)
                # centered accumulate: sum((r-mu)*r) = D*var
                sq_t = cx.tile([128, D], F32, tag="sq_t", name="sq_t", bufs=1)
                dvar = cx.tile([128, 1], F32, tag="dvar", name="dvar")
                nc.vector.scalar_tensor_tensor(
                    sq_t[:], r_t[:], negmu[:], r_t[:],
                    op0=ALU.add, op1=ALU.mult, accum_out=dvar[:])
                sd = cx.tile([128, 1], F32, tag="sd", name="sd")
                nc.scalar.activation(sd[:], dvar[:], AF.Sqrt,
                                     bias=eps_sb[:], scale=1.0 / D)
                rstd = cx.tile([128, 1], F32, tag="rstd", name="rstd")
                nc.vector.reciprocal(rstd[:], sd[:])
                x1 = cx1.tile([128, D], F16, tag=f"x1_{qs}", name=f"x1_{qs}")
                nc.vector.tensor_scalar(x1[:], r_t[:], negmu[:], rstd[:],
                                        op0=ALU.add, op1=ALU.mult)
                return x1

            def transpose_qs(x1_t, x1t_all, qs):
                # X1^T for one q-block: 6 PE transposes packed into one
                # psum bank, then a single strided DVE copy into x1t_all
                pt = cpt.tile([128, DB, 128], F16, tag="pt", name="pt")
                for d in range(DB):
                    nc.tensor.transpose(
                        pt[:, d, :],
                        x1_t[qs][:, d * 128:(d + 1) * 128], ident[:])
                nc.vector.tensor_copy(
                    x1t_all[:, :, qs * 128:(qs + 1) * 128], pt[:])

            def dense_qs(c, x1_t, x1t_all, qs):
                # proj (+bd via K=1 ones-row matmul) -> LN2 -> out rows
                pp = []
                for n0, nw in ((0, 384), (384, 384)):
                    p = cpp.tile([128, 384], F32, tag="pp", name="pp")
                    for d in range(DB):
                        nc.tensor.matmul(p[:],
                                         x1t_all[:, d, qs * 128:(qs + 1) * 128],
                                         wd_sb[d][:, n0:n0 + nw],
                                         start=(d == 0), stop=False)
                    nc.tensor.matmul(p[:], ones_sb[:], bd_sb[:, n0:n0 + nw],
                                     start=False, stop=True)
                    pp.append(p)
                y_t = cx.tile([128, D], F32, tag="y_t", name="y_t")
                t0 = cx.tile([128, 1], F32, tag="t0", name="t0")
                t1 = cx.tile([128, 1], F32, tag="t1", name="t1")
                nc.vector.scalar_tensor_tensor(
                    y_t[:, 0:384], pp[0][:], 0.0, x1_t[qs][:, 0:384],
                    op0=ALU.add, op1=ALU.add, accum_out=t0[:])
                nc.vector.scalar_tensor_tensor(
                    y_t[:, 384:D], pp[1][:], 0.0, x1_t[qs][:, 384:D],
                    op0=ALU.add, op1=ALU.add, accum_out=t1[:])
                negmu2 = cx.tile([128, 1], F32, tag="negmu2", name="negmu2")
                nc.vector.tensor_add(negmu2[:], t0[:], t1[:])
                nc.vector.tensor_scalar_mul(negmu2[:], negmu2[:], -1.0 / D)
                sq2 = cx.tile([128, D], F32, tag="sq2", name="sq2", bufs=1)
                dvar2 = cx.tile([128, 1], F32, tag="dvar2", name="dvar2")
                nc.vector.scalar_tensor_tensor(
                    sq2[:], y_t[:], negmu2[:], y_t[:],
                    op0=ALU.add, op1=ALU.mult, accum_out=dvar2[:])
                sd2 = cx.tile([128, 1], F32, tag="sd2", name="sd2")
                nc.scalar.activation(sd2[:], dvar2[:], AF.Sqrt,
                                     bias=eps_sb[:], scale=1.0 / D)
                rstd2 = cx.tile([128, 1], F32, tag="rstd2", name="rstd2")
                nc.vector.reciprocal(rstd2[:], sd2[:])
                out_t = cx.tile([128, D], F32, tag="out_t", name="out_t")
                nc.vector.tensor_scalar(out_t[:], y_t[:], negmu2[:],
                                        rstd2[:], op0=ALU.add, op1=ALU.mult)
                r0 = c * QCH + qs * 128
                nc.sync.dma_start(out_d[r0:r0 + 128, :], out_t[:])

            # software pipeline: interleave attn/transpose/dense per q-block
            # so PE matmuls cover every LN chain and every transpose->copy
            # latency; scores(c+1) covers the chunk's tail
            xres_cur = load_xres(0)
            et_cur = stage_scores(0)
            for c in range(NCH):
                if c + 1 < NCH:
                    xres_nxt = load_xres(c + 1)
                x1_t = [None] * nqb
                x1t_all = cx1t.tile([128, DB, QCH], F16, tag="x1t",
                                    name="x1t")
                x1_t[0] = attn_qs(c, et_cur, xres_cur, 0)
                x1_t[1] = attn_qs(c, et_cur, xres_cur, 1)
                transpose_qs(x1_t, x1t_all, 0)
                x1_t[2] = attn_qs(c, et_cur, xres_cur, 2)
                dense_qs(c, x1_t, x1t_all, 0)
                transpose_qs(x1_t, x1t_all, 1)
                x1_t[3] = attn_qs(c, et_cur, xres_cur, 3)
                dense_qs(c, x1_t, x1t_all, 1)
                transpose_qs(x1_t, x1t_all, 2)
                if c + 1 < NCH:
                    et_cur = stage_scores(c + 1)
                    xres_cur = xres_nxt
                dense_qs(c, x1_t, x1t_all, 2)
                transpose_qs(x1_t, x1t_all, 3)
                dense_qs(c, x1_t, x1t_all, 3)

    _split_matmul_waits(nc)
    nc.compile()
    return nc


_NC_CACHE = None


def _get_nc():
    global _NC_CACHE
    if _NC_CACHE is None:
        _NC_CACHE = _build()
    return _NC_CACHE


def _prep_in_maps(X, Wq, bq, Wk, bk, Wv, bv, Wd, bd):
    X = np.ascontiguousarray(X, np.float32)
    f16 = np.float16
    wq = np.asarray(Wq, np.float32).astype(f16).reshape(DB, 128, D)
    wk = np.asarray(Wk, np.float32).astype(f16).reshape(DB, 128, D)
    wv_aug = np.zeros((D, D + 2), f16)
    wv_aug[:, :D] = np.asarray(Wv, np.float32).astype(f16)
    wv_aug = wv_aug.reshape(DB, 128, D + 2)
    wd = np.asarray(Wd, np.float32).astype(f16).reshape(DB, 128, D)
    wkq = np.ascontiguousarray(np.concatenate([wk, wq], axis=2))
    wvd = np.ascontiguousarray(np.concatenate([wv_aug, wd], axis=2))
    bv_aug = np.zeros((1, D + 2), f16)
    bv_aug[0, :D] = np.asarray(bv, np.float32).astype(f16)
    bv_aug[0, D] = 1.0
    bd_r = np.asarray(bd, np.float32).astype(f16).reshape(1, D)
    shared = {
        "wkq": wkq, "wvd": wvd,
        "bqk": np.ascontiguousarray(np.concatenate(
            [np.asarray(bq, np.float32).reshape(DB, 128, 1),
             np.asarray(bk, np.float32).reshape(DB, 128, 1)], axis=0)
            .transpose(1, 0, 2).reshape(128, 2 * DB)),
        "bv": bv_aug, "bd": bd_r,
    }
    return [dict(shared,
                 xr=X[c].astype(f16),
                 xt=np.ascontiguousarray(X[c].T).astype(f16).reshape(DB, 128, S))
            for c in range(N_CORES)]


def _run(inputs, trace=False, trace_kwargs=None):
    in_maps = _prep_in_maps(
        inputs["X"], inputs["Wq"], inputs["bq"], inputs["Wk"], inputs["bk"],
        inputs["Wv"], inputs["bv"], inputs["Wd"], inputs["bd"])
    nc = _get_nc()
    res = run_bass_kernel_spmd(nc, in_maps, list(range(N_CORES)),
                               trace=trace, **(trace_kwargs or {}))
    out = np.stack([res.results[c]["out"] for c in range(N_CORES)])
    return out, res


def kernel(X, Wq, bq, Wk, bk, Wv, bv, Wd, bd, g1, b1, g2, b2):
    out, _ = _run(dict(X=X, Wq=Wq, bq=bq, Wk=Wk, bk=bk, Wv=Wv, bv=bv,
                       Wd=Wd, bd=bd))
    g1 = np.asarray(g1); b1 = np.asarray(b1)
    g2 = np.asarray(g2); b2 = np.asarray(b2)
    # The kernel folds the (identity) LN affines away; handle the general
    # case anyway. A non-identity g1/b1 feeds the dense layer and cannot be
    # patched after the fact -> recompute on host (never hit for this
    # problem's deterministic inputs: g=1, b=0).
    if not (np.allclose(g1, 1.0) and np.allclose(b1, 0.0)):
        return _host_reference(X, Wq, bq, Wk, bk, Wv, bv, Wd, bd, g1, b1, g2, b2)
    if not (np.allclose(g2, 1.0) and np.allclose(b2, 0.0)):
        out = out * np.asarray(g2) + np.asarray(b2)
    return out.astype(np.float32)


def _host_reference(X, Wq, bq, Wk, bk, Wv, bv, Wd, bd, g1, b1, g2, b2):
    X = np.asarray(X, np.float64)
    out = np.empty_like(X)
    for c in range(X.shape[0]):
        x = X[c]
        Q = np.maximum(x @ Wq + bq, 0)
        K = np.maximum(x @ Wk + bk, 0)
        V = np.maximum(x @ Wv + bv, 0)
        Sc = Q @ K.T
        Sc -= Sc.max(-1, keepdims=True)
        E = np.exp(Sc)
        A = (E @ V) / E.sum(-1, keepdims=True)
        X1 = x + A
        X1 = (X1 - X1.mean(-1, keepdims=True)) / np.sqrt(
            X1.var(-1, keepdims=True) + EPS) * g1 + b1
        X2 = X1 + X1 @ Wd + bd
        X2 = (X2 - X2.mean(-1, keepdims=True)) / np.sqrt(
            X2.var(-1, keepdims=True) + EPS) * g2 + b2
        out[c] = X2
    return out.astype(np.float32)


# revision 21
# speedup vs baseline: 1.4388x; 1.0608x over previous
"""Trainium2 Bass kernel for an 8-batch single-head attention block.

Reference computation (per batch b of 8, S=2048 seq, D=A=768):
    Q = relu(X Wq + bq); K = relu(X Wk + bk); V = relu(X Wv + bv)
    P = softmax(Q K^T)          (no 1/sqrt(d) scale)
    X1 = LN(X + P V)
    X2 = LN(X1 + X1 Wd + bd)    (LN affines are identity in this problem)

Sharding: data-parallel — batch b -> NeuronCore b (8 cores, no collectives).

v2 design notes (from the v1 trace):
  * v1 ran all matmuls in fp32r. The PE multiplies bf16/fp16/fp32r all at
    the same FP22 internal precision and 1 cycle/row, but fp32r weight
    loads (4B) take ~213ns vs ~107ns for 16-bit — with an LDWEIGHTS per
    matmul the PE was LDW-throughput-bound (~265ns issue period vs the
    213ns stream floor). fp16 keeps fp32r-level precision (11-bit
    mantissa -> FP22 exactly) at half the LDW/DMA cost, so v2 stores every
    matmul operand in fp16 (E^T stays bf16 for exp range; V bf16 to match).
  * Q^T is kept resident in SBUF (no DRAM spill round-trip) -> kills the
    ~6us phase-boundary stall + HAM re-throttle.
  * Biases bv/bd are folded into the matmul accumulation groups as K=1
    matmuls against a ones row (removes DVE bias adds from the critical
    path); bq/bk ride the ACT relu bias port as before.
  * LayerNorm math runs on DVE via tensor_scalar (two scalar operands), a
    centered-accumulate trick gets D*var in the same pass that computes
    the residual sum; ACT only does Relu / Exp / Sqrt -> far fewer
    activation-table reloads (~1.5us each on v1).
  * Phase C is software-pipelined: per chunk, scores(c+1) is issued
    between attn(c) and transpose/dense(c) so the PE chews on scores
    while the DVE LayerNorm chain for chunk c completes.
"""

from contextlib import ExitStack

import numpy as np

import concourse.bass as bass
import concourse.mybir as mybir
import concourse.tile as tile
from concourse import bacc
from concourse.bass_utils import run_bass_kernel_spmd
from concourse.masks import make_identity

S, D = 2048, 768
N_CORES = 8
SB, DB = S // 128, D // 128  # 16 s-blocks, 6 d-blocks
SCH = 512   # phase-B s-chunk width
QCH = 512   # phase-C q-chunk width
NCH = S // SCH
F32 = mybir.dt.float32
F16 = mybir.dt.float16
BF16 = mybir.dt.bfloat16
ET_DT = BF16
V_DT = BF16
AF = mybir.ActivationFunctionType
ALU = mybir.AluOpType
EPS = 1e-5


def _split_matmul_waits(nc):
    """Walrus allows only one semaphore wait on self-loading (fp32/fp32r/
    transpose) Matmult instructions; move extra waits onto a preceding
    InstEventSemaphore (which may carry two waits each)."""
    for bb in nc.main_func.blocks:
        new_insts = []
        for inst in bb.instructions:
            if isinstance(inst, mybir.InstMatmult) and inst.sync_info is not None \
                    and len(inst.sync_info.on_wait) > 1:
                waits = list(inst.sync_info.on_wait)
                extra, keep = waits[:-1], waits[-1:]
                while extra:
                    chunk, extra = extra[:2], extra[2:]
                    ev = mybir.InstEventSemaphore(
                        name=nc.get_next_instruction_name(), ins=[], outs=[])
                    ev.engine = inst.engine
                    ev.sync_info = mybir.SyncInfo(on_wait=chunk, on_update=[])
                    nc.register_instruction(ev)
                    new_insts.append(ev)
                inst.sync_info = mybir.SyncInfo(
                    on_wait=keep, on_update=list(inst.sync_info.on_update))
            new_insts.append(inst)
        bb.instructions[:] = new_insts


def _build():
    nc = bacc.Bacc("TRN2", target_bir_lowering=False, debug=False,
                   enable_asserts=False, num_devices=N_CORES)

    # weights packed per d-block into wide rows (3KB lines DMA much faster
    # than 1.5KB): wkq = wk|wq, wvd = wv_aug|wd
    xt_d = nc.dram_tensor("xt", [DB, 128, S], F16, kind="ExternalInput").ap()
    xr_d = nc.dram_tensor("xr", [S, D], F16, kind="ExternalInput").ap()
    wkq_d = nc.dram_tensor("wkq", [DB, 128, 2 * D], F16, kind="ExternalInput").ap()
    wvd_d = nc.dram_tensor("wvd", [DB, 128, 2 * D + 2], F16,
                           kind="ExternalInput").ap()
    bqk_d = nc.dram_tensor("bqk", [128, 2 * DB], F32, kind="ExternalInput").ap()
    bv_d = nc.dram_tensor("bv", [1, D + 2], F16, kind="ExternalInput").ap()
    bd_d = nc.dram_tensor("bd", [1, D], F16, kind="ExternalInput").ap()
    out_d = nc.dram_tensor("out", [S, D], F32, kind="ExternalOutput").ap()

    with tile.TileContext(nc) as tc, ExitStack() as ctx:
        consts = ctx.enter_context(tc.tile_pool(name="consts", bufs=1))
        pers = ctx.enter_context(tc.tile_pool(name="pers", bufs=1))
        wdp = ctx.enter_context(tc.tile_pool(name="wdp", bufs=1))

        # critical-path DMAs first on the sync queue: bqk, wk, xt chunk 0
        bqk_sb = consts.tile([128, 2 * DB], F32, tag="bqk", name="bqk")
        nc.sync.dma_start(bqk_sb[:], bqk_d[:])
        bq_sb = [bqk_sb[:, e:e + 1] for e in range(DB)]
        bk_sb = [bqk_sb[:, DB + e:DB + e + 1] for e in range(DB)]

        ident = consts.tile([128, 128], F16, tag="ident", name="ident")
        make_identity(nc, ident[:])
        eps_sb = consts.tile([128, 1], F32, tag="eps", name="eps")
        nc.gpsimd.memset(eps_sb[:], EPS)
        # bv/bd tiles allocated here; their DMAs are issued after the
        # startup-critical xt chunk 0 loads (same scalar queue). Both are
        # partition-broadcast by gpsimd into full tiles for DVE adds.
        bv_sb = consts.tile([1, D + 2], F16, tag="bv", name="bv")
        bd_sb = consts.tile([1, D], F16, tag="bd", name="bd")
        bv_bc = consts.tile([128, D + 2], F16, tag="bv_bc", name="bv_bc")
        bd_bc = consts.tile([128, D], F16, tag="bd_bc", name="bd_bc")
        # wv|wd packed tiles persist through phase C (wd slices used there)
        wvd_sb = [wdp.tile([128, 2 * D + 2], F16, tag=f"wvd{d}",
                           name=f"wvd{d}") for d in range(DB)]
        wv_sb = [t[:, 0:D + 2] for t in wvd_sb]
        wd_sb = [t[:, D + 2:2 * D + 2] for t in wvd_sb]

        # ---------------- Phase B: K^T, Q^T (both resident), V (resident)
        kt = {}    # (e, chunk) -> [128, SCH] f16 tile
        qt = {}    # (e, chunk) -> [128, SCH] f16 tile
        v_sb = []  # k-block -> [128, 770] bf16 tile
        for k in range(SB):
            v_sb.append(pers.tile([128, D + 2], V_DT, tag=f"v{k}", name=f"v{k}"))

        with tc.tile_pool(name="wqkv", bufs=1) as wpool, \
             tc.tile_pool(name="bx", bufs=2) as bx, \
             tc.tile_pool(name="bkq", bufs=1, space="PSUM") as bkq, \
             tc.tile_pool(name="bpm", bufs=2, space="PSUM") as bpm:
            # parallel DMA queues at startup: wk|wq packed tiles on sync,
            # xt chunk 0 on scalar -> first matmul issues after ~2 tiles
            wkq_sb, xt_first = [], []
            for d in range(DB):
                t = wpool.tile([128, 2 * D], F16, tag=f"wkq{d}", name=f"wkq{d}")
                nc.sync.dma_start(t[:], wkq_d[d])
                wkq_sb.append(t)
                t = bx.tile([128, SCH], F16, tag=f"xt{d}", name=f"xt{d}")
                nc.scalar.dma_start(t[:], xt_d[d, :, 0:SCH])
                xt_first.append(t)
            wk_sb = [t[:, 0:D] for t in wkq_sb]
            wq_sb = [t[:, D:2 * D] for t in wkq_sb]
            for d in range(DB):
                nc.sync.dma_start(wvd_sb[d][:], wvd_d[d])
            nc.scalar.dma_start(bv_sb[:], bv_d[:])
            nc.scalar.dma_start(bd_sb[:], bd_d[:])
            nc.gpsimd.partition_broadcast(bv_bc[:], bv_sb[:])
            nc.gpsimd.partition_broadcast(bd_bc[:], bd_sb[:])

            nsb = SCH // 128  # s-blocks per chunk
            for c in range(NCH):
                if c == 0:
                    xt_c = xt_first
                else:
                    xt_c = []
                    for d in range(DB):
                        t = bx.tile([128, SCH], F16, tag=f"xt{d}", name=f"xt{d}")
                        nc.sync.dma_start(t[:], xt_d[d, :, c * SCH:(c + 1) * SCH])
                        xt_c.append(t)
                # K^T and Q^T e-blocks, d-OUTER with one psum bank per e:
                # matmuls pace with the weight/xt DMA tile arrivals instead
                # of waiting for the full weight set (chunk-0 startup)
                for which in ("k", "q"):
                    w_sb, b_sb = (wk_sb, bk_sb) if which == "k" else (wq_sb, bq_sb)
                    ps = [bkq.tile([128, SCH], F32, tag=f"pe{e}", name=f"pe{e}")
                          for e in range(DB)]
                    for d in range(DB):
                        for e in range(DB):
                            nc.tensor.matmul(ps[e][:],
                                             w_sb[d][:, e * 128:(e + 1) * 128],
                                             xt_c[d][:], start=(d == 0),
                                             stop=(d == DB - 1))
                    for e in range(DB):
                        t = pers.tile([128, SCH], F16, tag=f"{which}t{e}_{c}",
                                      name=f"{which}t{e}_{c}")
                        nc.scalar.activation(t[:], ps[e][:], AF.Relu,
                                             bias=b_sb[e])
                        (kt if which == "k" else qt)[(e, c)] = t
                # V s-blocks (col 768 == 1.0 via bv_aug for softmax row-sums;
                # bv added on DVE from the partition-broadcast row)
                for sb in range(nsb):
                    k_idx = c * nsb + sb
                    for n0, nw in ((0, 512), (512, D + 2 - 512)):
                        pv = bpm.tile([128, 512], F32, tag="pmm", name="pmm")
                        for d in range(DB):
                            nc.tensor.matmul(pv[:, :nw],
                                             xt_c[d][:, sb * 128:(sb + 1) * 128],
                                             wv_sb[d][:, n0:n0 + nw],
                                             start=(d == 0), stop=(d == DB - 1))
                        vt = bx.tile([128, 512], F16, tag="vt", name="vt")
                        nc.vector.tensor_add(vt[:, :nw], pv[:, :nw],
                                             bv_bc[:, n0:n0 + nw])
                        nc.scalar.activation(v_sb[k_idx][:, n0:n0 + nw],
                                             vt[:, :nw], AF.Relu)

        # ------- Phase C (pipelined): scores -> exp -> attn -> LN1 -> proj -> LN2
        nqb = QCH // 128   # q-blocks per chunk
        kt_per_chunk = SCH // 128
        with tc.tile_pool(name="cx", bufs=2) as cx, \
             tc.tile_pool(name="cxr", bufs=2) as cxr, \
             tc.tile_pool(name="cx1", bufs=2) as cx1, \
             tc.tile_pool(name="cx1t", bufs=2) as cx1t, \
             tc.tile_pool(name="cet", bufs=2) as cet, \
             tc.tile_pool(name="cst", bufs=2, space="PSUM") as cst, \
             tc.tile_pool(name="cpa0", bufs=2, space="PSUM") as cpa0, \
             tc.tile_pool(name="cpa1", bufs=1, space="PSUM") as cpa1, \
             tc.tile_pool(name="cpt", bufs=1, space="PSUM") as cpt, \
             tc.tile_pool(name="cpp", bufs=2, space="PSUM") as cpp:

            def load_xres(c):
                x_res = []
                for qs in range(nqb):
                    t = cxr.tile([128, D], F16, tag=f"xr{qs}", name=f"xr{qs}")
                    nc.scalar.dma_start(t[:], xr_d[c * QCH + qs * 128:
                                                   c * QCH + (qs + 1) * 128, :])
                    x_res.append(t)
                return x_res

            def stage_scores(c):
                """S^T = K Q^T per k-block -> E^T = exp(S^T) bf16 (no max
                subtraction: scores < ~72 so exp stays in fp32 range)."""
                et = []
                for k in range(SB):
                    pst = cst.tile([128, QCH], F32, tag="pst", name="pst")
                    for e in range(DB):
                        nc.tensor.matmul(
                            pst[:],
                            kt[(e, k // kt_per_chunk)][
                                :, (k % kt_per_chunk) * 128:
                                   (k % kt_per_chunk + 1) * 128],
                            qt[(e, c)][:], start=(e == 0), stop=(e == DB - 1))
                    et_t = cet.tile([128, QCH], ET_DT, tag=f"et{k}", name=f"et{k}")
                    nc.scalar.activation(et_t[:], pst[:], AF.Exp)
                    et.append(et_t)
                return et

            def attn_qs(c, et, x_res, qs):
                """attn rows + row-sum (col 768) in PSUM for one q-block;
                normalize + residual + LayerNorm -> X1 (fp16, SBUF)."""
                pa0 = cpa0.tile([128, 512], F32, tag="pa0", name="pa0")
                pa1 = cpa1.tile([128, D + 2 - 512], F32, tag="pa1", name="pa1")
                for k in range(SB):
                    nc.tensor.matmul(pa0[:],
                                     et[k][:, qs * 128:(qs + 1) * 128],
                                     v_sb[k][:, 0:512],
                                     start=(k == 0), stop=(k == SB - 1))
                for k in range(SB):
                    nc.tensor.matmul(pa1[:],
                                     et[k][:, qs * 128:(qs + 1) * 128],
                                     v_sb[k][:, 512:D + 2],
                                     start=(k == 0), stop=(k == SB - 1))
                rcp = cx.tile([128, 1], F32, tag="rcp", name="rcp")
                nc.vector.reciprocal(rcp[:], pa1[:, 256:257])
                r_t = cx.tile([128, D], F32, tag="r_t", name="r_t")
                s0 = cx.tile([128, 1], F32, tag="s0", name="s0")
                s1 = cx.tile([128, 1], F32, tag="s1", name="s1")
                nc.vector.scalar_tensor_tensor(
                    r_t[:, 0:512], pa0[:], rcp[:], x_res[qs][:, 0:512],
                    op0=ALU.mult, op1=ALU.add, accum_out=s0[:])
                nc.vector.scalar_tensor_tensor(
                    r_t[:, 512:D], pa1[:, 0:256], rcp[:], x_res[qs][:, 512:D],
                    op0=ALU.mult, op1=ALU.add, accum_out=s1[:])
                negmu = cx.tile([128, 1], F32, tag="negmu", name="negmu")
                nc.vector.tensor_add(negmu[:], s0[:], s1[:])
                nc.vector.tensor_scalar_mul(negmu[:], negmu[:], -1.0 / D)
                # centered accumulate: sum((r-mu)*r) = D*var
                sq_t = cx.tile([128, D], F32, tag="sq_t", name="sq_t", bufs=1)
                dvar = cx.tile([128, 1], F32, tag="dvar", name="dvar")
                nc.vector.scalar_tensor_tensor(
                    sq_t[:], r_t[:], negmu[:], r_t[:],
                    op0=ALU.add, op1=ALU.mult, accum_out=dvar[:])
                sd = cx.tile([128, 1], F32, tag="sd", name="sd")
                nc.scalar.activation(sd[:], dvar[:], AF.Sqrt,
                                     bias=eps_sb[:], scale=1.0 / D)
                rstd = cx.tile([128, 1], F32, tag="rstd", name="rstd")
                nc.vector.reciprocal(rstd[:], sd[:])
                x1 = cx1.tile([128, D], F16, tag=f"x1_{qs}", name=f"x1_{qs}")
                nc.vector.tensor_scalar(x1[:], r_t[:], negmu[:], rstd[:],
                                        op0=ALU.add, op1=ALU.mult)
                return x1

            def transpose_qs(x1_t, x1t_all, qs):
                # X1^T for one q-block: 6 PE transposes packed into one
                # psum bank, then a single strided DVE copy into x1t_all
                pt = cpt.tile([128, DB, 128], F16, tag="pt", name="pt")
                for d in range(DB):
                    nc.tensor.transpose(
                        pt[:, d, :],
                        x1_t[qs][:, d * 128:(d + 1) * 128], ident[:])
                nc.vector.tensor_copy(
                    x1t_all[:, :, qs * 128:(qs + 1) * 128], pt[:])

            def dense_qs(c, x1_t, x1t_all, qs):
                # proj -> +x1+bd residual -> LN2 -> out rows
                pp = []
                for n0, nw in ((0, 384), (384, 384)):
                    p = cpp.tile([128, 384], F32, tag="pp", name="pp")
                    for d in range(DB):
                        nc.tensor.matmul(p[:],
                                         x1t_all[:, d, qs * 128:(qs + 1) * 128],
                                         wd_sb[d][:, n0:n0 + nw],
                                         start=(d == 0), stop=(d == DB - 1))
                    pp.append(p)
                x1bd = cx.tile([128, D], F16, tag="x1bd", name="x1bd")
                nc.vector.tensor_add(x1bd[:], x1_t[qs][:], bd_bc[:])
                y_t = cx.tile([128, D], F32, tag="y_t", name="y_t")
                t0 = cx.tile([128, 1], F32, tag="t0", name="t0")
                t1 = cx.tile([128, 1], F32, tag="t1", name="t1")
                nc.vector.scalar_tensor_tensor(
                    y_t[:, 0:384], pp[0][:], 0.0, x1bd[:, 0:384],
                    op0=ALU.add, op1=ALU.add, accum_out=t0[:])
                nc.vector.scalar_tensor_tensor(
                    y_t[:, 384:D], pp[1][:], 0.0, x1bd[:, 384:D],
                    op0=ALU.add, op1=ALU.add, accum_out=t1[:])
                negmu2 = cx.tile([128, 1], F32, tag="negmu2", name="negmu2")
                nc.vector.tensor_add(negmu2[:], t0[:], t1[:])
                nc.vector.tensor_scalar_mul(negmu2[:], negmu2[:], -1.0 / D)
                sq2 = cx.tile([128, D], F32, tag="sq2", name="sq2", bufs=1)
                dvar2 = cx.tile([128, 1], F32, tag="dvar2", name="dvar2")
                nc.vector.scalar_tensor_tensor(
                    sq2[:], y_t[:], negmu2[:], y_t[:],
                    op0=ALU.add, op1=ALU.mult, accum_out=dvar2[:])
                sd2 = cx.tile([128, 1], F32, tag="sd2", name="sd2")
                nc.scalar.activation(sd2[:], dvar2[:], AF.Sqrt,
                                     bias=eps_sb[:], scale=1.0 / D)
                rstd2 = cx.tile([128, 1], F32, tag="rstd2", name="rstd2")
                nc.vector.reciprocal(rstd2[:], sd2[:])
                out_t = cx.tile([128, D], F32, tag="out_t", name="out_t")
                nc.vector.tensor_scalar(out_t[:], y_t[:], negmu2[:],
                                        rstd2[:], op0=ALU.add, op1=ALU.mult)
                r0 = c * QCH + qs * 128
                nc.sync.dma_start(out_d[r0:r0 + 128, :], out_t[:])

            # software pipeline: interleave attn/transpose/dense per q-block
            # so PE matmuls cover every LN chain and every transpose->copy
            # latency; scores(c+1) covers the chunk's tail
            xres_cur = load_xres(0)
            et_cur = stage_scores(0)
            for c in range(NCH):
                if c + 1 < NCH:
                    xres_nxt = load_xres(c + 1)
                x1_t = [None] * nqb
                x1t_all = cx1t.tile([128, DB, QCH], F16, tag="x1t",
                                    name="x1t")
                x1_t[0] = attn_qs(c, et_cur, xres_cur, 0)
                x1_t[1] = attn_qs(c, et_cur, xres_cur, 1)
                transpose_qs(x1_t, x1t_all, 0)
                x1_t[2] = attn_qs(c, et_cur, xres_cur, 2)
                dense_qs(c, x1_t, x1t_all, 0)
                transpose_qs(x1_t, x1t_all, 1)
                x1_t[3] = attn_qs(c, et_cur, xres_cur, 3)
                dense_qs(c, x1_t, x1t_all, 1)
                transpose_qs(x1_t, x1t_all, 2)
                if c + 1 < NCH:
                    et_cur = stage_scores(c + 1)
                    xres_cur = xres_nxt
                dense_qs(c, x1_t, x1t_all, 2)
                transpose_qs(x1_t, x1t_all, 3)
                dense_qs(c, x1_t, x1t_all, 3)

    _split_matmul_waits(nc)
    nc.compile()
    return nc


_NC_CACHE = None


def _get_nc():
    global _NC_CACHE
    if _NC_CACHE is None:
        _NC_CACHE = _build()
    return _NC_CACHE


def _prep_in_maps(X, Wq, bq, Wk, bk, Wv, bv, Wd, bd):
    X = np.ascontiguousarray(X, np.float32)
    f16 = np.float16
    wq = np.asarray(Wq, np.float32).astype(f16).reshape(DB, 128, D)
    wk = np.asarray(Wk, np.float32).astype(f16).reshape(DB, 128, D)
    wv_aug = np.zeros((D, D + 2), f16)
    wv_aug[:, :D] = np.asarray(Wv, np.float32).astype(f16)
    wv_aug = wv_aug.reshape(DB, 128, D + 2)
    wd = np.asarray(Wd, np.float32).astype(f16).reshape(DB, 128, D)
    wkq = np.ascontiguousarray(np.concatenate([wk, wq], axis=2))
    wvd = np.ascontiguousarray(np.concatenate([wv_aug, wd], axis=2))
    bv_aug = np.zeros((1, D + 2), f16)
    bv_aug[0, :D] = np.asarray(bv, np.float32).astype(f16)
    bv_aug[0, D] = 1.0
    bd_r = np.asarray(bd, np.float32).astype(f16).reshape(1, D)
    shared = {
        "wkq": wkq, "wvd": wvd,
        "bqk": np.ascontiguousarray(np.concatenate(
            [np.asarray(bq, np.float32).reshape(DB, 128, 1),
             np.asarray(bk, np.float32).reshape(DB, 128, 1)], axis=0)
            .transpose(1, 0, 2).reshape(128, 2 * DB)),
        "bv": bv_aug, "bd": bd_r,
    }
    return [dict(shared,
                 xr=X[c].astype(f16),
                 xt=np.ascontiguousarray(X[c].T).astype(f16).reshape(DB, 128, S))
            for c in range(N_CORES)]


def _run(inputs, trace=False, trace_kwargs=None):
    in_maps = _prep_in_maps(
        inputs["X"], inputs["Wq"], inputs["bq"], inputs["Wk"], inputs["bk"],
        inputs["Wv"], inputs["bv"], inputs["Wd"], inputs["bd"])
    nc = _get_nc()
    res = run_bass_kernel_spmd(nc, in_maps, list(range(N_CORES)),
                               trace=trace, **(trace_kwargs or {}))
    out = np.stack([res.results[c]["out"] for c in range(N_CORES)])
    return out, res


def kernel(X, Wq, bq, Wk, bk, Wv, bv, Wd, bd, g1, b1, g2, b2):
    out, _ = _run(dict(X=X, Wq=Wq, bq=bq, Wk=Wk, bk=bk, Wv=Wv, bv=bv,
                       Wd=Wd, bd=bd))
    g1 = np.asarray(g1); b1 = np.asarray(b1)
    g2 = np.asarray(g2); b2 = np.asarray(b2)
    # The kernel folds the (identity) LN affines away; handle the general
    # case anyway. A non-identity g1/b1 feeds the dense layer and cannot be
    # patched after the fact -> recompute on host (never hit for this
    # problem's deterministic inputs: g=1, b=0).
    if not (np.allclose(g1, 1.0) and np.allclose(b1, 0.0)):
        return _host_reference(X, Wq, bq, Wk, bk, Wv, bv, Wd, bd, g1, b1, g2, b2)
    if not (np.allclose(g2, 1.0) and np.allclose(b2, 0.0)):
        out = out * np.asarray(g2) + np.asarray(b2)
    return out.astype(np.float32)


def _host_reference(X, Wq, bq, Wk, bk, Wv, bv, Wd, bd, g1, b1, g2, b2):
    X = np.asarray(X, np.float64)
    out = np.empty_like(X)
    for c in range(X.shape[0]):
        x = X[c]
        Q = np.maximum(x @ Wq + bq, 0)
        K = np.maximum(x @ Wk + bk, 0)
        V = np.maximum(x @ Wv + bv, 0)
        Sc = Q @ K.T
        Sc -= Sc.max(-1, keepdims=True)
        E = np.exp(Sc)
        A = (E @ V) / E.sum(-1, keepdims=True)
        X1 = x + A
        X1 = (X1 - X1.mean(-1, keepdims=True)) / np.sqrt(
            X1.var(-1, keepdims=True) + EPS) * g1 + b1
        X2 = X1 + X1 @ Wd + bd
        X2 = (X2 - X2.mean(-1, keepdims=True)) / np.sqrt(
            X2.var(-1, keepdims=True) + EPS) * g2 + b2
        out[c] = X2
    return out.astype(np.float32)
